# revision 1
# baseline (speedup 1.0000x reference)
"""Trainium2 Bass kernel for nn_MultiHeadAttention (B=2, S=2048, D=1024, H=16).

Sharding: 8 cores = 2 (batch) x 4 (head groups of 4 heads / 256 proj dims).
Each core computes q/k/v projections for its 256-dim slice, attention for its
4 heads, and a partial out-projection y_part = attn_out @ Wo[slice].  The host
gather sums the 4 partials per batch (bo is added on one core per group via a
zeros-bias trick so the program stays SPMD-uniform).

Kernel layout choices (see comments inline):
 - x is transposed once on the PE (d on partitions) since all projections
   contract over d.
 - Q,K are produced transposed ([n, s]); scores are computed transposed
   ([k, q]) so the softmax denominator never needs an on-chip transpose.
 - Attention-weight @ V is col-packed (two heads concurrently on the 128x128
   PE array via tile_position), row-sums of exp come from M=1 ones-matmuls.
 - Normalization by 1/rowsum happens on the PV output (psum) with a
   DMA-broadcast reciprocal; out-projection reads the natural outT layout.
"""

import sys

sys.path.insert(0, "/opt/trn_rl_repo")

import numpy as np

import concourse.bass as bass
import concourse.mybir as mybir
import concourse.tile as _tile_mod
from concourse.masks import make_identity
from concourse.tile import TileContext
from concourse.vector_clock import ScopedClock


def _drain_and_barrier_split_waits(self, tick_clock, wait_clock):
    """Replacement for TileContext._drain_and_barrier.

    The walrus build in this container only accepts one sync-wait command per
    CTRL instruction; the stock tail drain carries one wait per outstanding
    proc and fails codegen with "Too many sync wait commands".  Attach the
    waits to a nop first, then redistribute the surplus onto extra nops.
    """
    carrier = self.nc.sync.nop()
    wait_clock.add_sem_waits(carrier.ins, ScopedClock({None: tick_clock.global_clock}))
    si = carrier.ins.sync_info
    if si is not None and len(si.on_wait) > 1:
        waits = list(si.on_wait)
        carrier.ins.sync_info = mybir.SyncInfo(
            on_wait=[waits[0]], on_update=list(si.on_update)
        )
        for w in waits[1:]:
            extra = self.nc.sync.nop()
            extra.ins.sync_info = mybir.SyncInfo(on_wait=[w], on_update=[])
    self.nc.sync.drain()

    self.nc.all_engine_barrier()
    assert self.sems is not None
    popped = self.nc._tile_sem_poison_stack.pop()
    assert popped is self._sem_poison
    self.nc.clear_and_free_semaphores(list(self.sems.allocated().values()))
    self.nc.all_engine_barrier()


_tile_mod.TileContext._drain_and_barrier = _drain_and_barrier_split_waits




def _split_excess_waits(nc):
    """This container's walrus accepts only ONE sync-wait command per
    instruction.  Tile emits up to 3.  Hoist all but the last wait of each
    instruction onto fresh same-engine NoOps placed directly before it --
    sound because walrus lowers DMA waits into the issuing sequencer's
    pseudo-instruction, so waits always gate the same sequencer stream."""
    ctr = 0
    for fn in nc.m.functions:
        for blk in fn.blocks:
            rewritten = []
            changed = False
            for ins in blk.instructions:
                si = ins.sync_info
                if si is not None and len(si.on_wait) > 1:
                    waits = list(si.on_wait)
                    for w in waits[:-1]:
                        nop = mybir.InstNoOp(name=f"I-wsplit-{ctr}", ins=[], outs=[])
                        ctr += 1
                        nop.engine = ins.engine
                        nop.sync_info = mybir.SyncInfo(on_wait=[w], on_update=[])
                        nc.register_instruction(nop)
                        rewritten.append(nop)
                    ins.sync_info = mybir.SyncInfo(
                        on_wait=[waits[-1]], on_update=list(si.on_update)
                    )
                    changed = True
                rewritten.append(ins)
            if changed:
                blk.instructions = rewritten
    return nc

F32 = mybir.dt.float32
BF16 = mybir.dt.bfloat16
ADD = mybir.AluOpType.add
MULT = mybir.AluOpType.mult
EXP = mybir.ActivationFunctionType.Exp

P = 128
D_MODEL = 1024
N_HEADS = 16
HEAD_DIM = 64
SCALE = HEAD_DIM**-0.5

# per-core sizes
NL = 256  # local projection dims (4 heads x 64)
HL = 4  # local heads
QBS = 512  # q block size for attention


def build_bass(S: int) -> bass.Bass:
    """One SPMD program; every core runs it on its own shard."""
    D = D_MODEL
    DC = D // P  # d chunks (8)
    SC = S // P  # s chunks
    QB = S // QBS  # q blocks
    KC = S // P  # k chunks

    nc = bass.Bass()
    x = nc.declare_dram_parameter("x", [S, D], F32, isOutput=False)
    wq = nc.declare_dram_parameter("wq", [D, NL], F32, isOutput=False)
    wk = nc.declare_dram_parameter("wk", [D, NL], F32, isOutput=False)
    wv = nc.declare_dram_parameter("wv", [D, NL], F32, isOutput=False)
    bq = nc.declare_dram_parameter("bq", [NL], F32, isOutput=False)
    bk = nc.declare_dram_parameter("bk", [NL], F32, isOutput=False)
    bv = nc.declare_dram_parameter("bv", [NL], F32, isOutput=False)
    wo = nc.declare_dram_parameter("wo", [NL, D], F32, isOutput=False)
    bo = nc.declare_dram_parameter("bo", [D], F32, isOutput=False)
    y = nc.declare_dram_parameter("y", [S, D], F32, isOutput=True)

    with TileContext(nc) as tc:
        with (
            tc.tile_pool(name="persist", bufs=1) as pp,
            tc.tile_pool(name="stage", bufs=3) as stage,
            tc.tile_pool(name="expp", bufs=3) as expp,
            tc.tile_pool(name="small", bufs=3) as small,
        ):
            # ---- constants / biases ----
            ident = pp.tile([P, P], F32, name="ident")
            make_identity(nc, ident)
            ones = pp.tile([P, HEAD_DIM], BF16, name="ones")
            nc.vector.memset(ones, 1.0)

            # ---- persistent activations ----
            xT = pp.tile([P, DC, S], BF16, name="xT")  # [d_in_chunk, dc, s]
            QT = pp.tile([P, 2, S], BF16, name="QT")  # [n_in_chunk, nchunk, s]
            KT = pp.tile([P, 2, S], BF16, name="KT")
            V = pp.tile([P, SC, HL, HEAD_DIM], BF16, name="V")  # [s_in_chunk, sc, h, dh]
            outT = pp.tile([P, 2, S], BF16, name="outT")  # [n_in_chunk, hp, q]

            # ---- phase A: x load + PE transpose + KT + QT[qb=0] ----
            with tc.tile_pool(name="psA", bufs=1, space="PSUM") as psA:
                for sg in range(SC // 4):  # groups of 4 s-chunks
                    xts = []
                    for j in range(4):
                        xt = stage.tile([P, D], F32, tag="x", bufs=6)
                        eng = (nc.sync, nc.scalar, nc.gpsimd, nc.scalar)[j]
                        eng.dma_start(xt, x[(sg * 4 + j) * P : (sg * 4 + j + 1) * P, :])
                        xts.append(xt)
                    for dc in range(DC):
                        tp = psA.tile([P, 4, P], F32, tag="tp", bufs=2)
                        for j in range(4):
                            nc.tensor.transpose(tp[:, j, :], xts[j][:, dc * P : (dc + 1) * P], ident)
                        nc.vector.tensor_copy(xT[:, dc, sg * 512 : (sg + 1) * 512], tp)

                bq_sb = pp.tile([P, 2], F32, name="bq_sb")
                nc.sync.dma_start(bq_sb, bq[:].rearrange("(o p) -> p o", p=P))
                bk_sb = pp.tile([P, 2], F32, name="bk_sb")
                nc.sync.dma_start(bk_sb, bk[:].rearrange("(o p) -> p o", p=P))
                bv_sb = pp.tile([P, NL], F32, name="bv_sb")
                nc.sync.dma_start(bv_sb, bv[:].unsqueeze(0).to_broadcast((P, NL)))
                bo_sb = pp.tile([P, D], F32, name="bo_sb")
                nc.sync.dma_start(bo_sb, bo[:].unsqueeze(0).to_broadcast((P, D)))

                # ---- weights -> bf16 ----
                wq_bf = pp.tile([P, DC, NL], BF16, name="wq_bf")
                wk_bf = pp.tile([P, DC, NL], BF16, name="wk_bf")
                wv_bf = pp.tile([P, DC, NL], BF16, name="wv_bf")
                for w_dram, w_bf in ((wq, wq_bf), (wk, wk_bf), (wv, wv_bf)):
                    w3 = w_dram[:].rearrange("(c p) n -> p c n", p=P)
                    for dc in range(DC):
                        wst = stage.tile([P, NL], F32, tag="wst")
                        nc.sync.dma_start(wst, w3[:, dc, :])
                        nc.vector.tensor_copy(w_bf[:, dc, :], wst)
                wo_bf = pp.tile([P, 2, D], BF16, name="wo_bf")
                wo3 = wo[:].rearrange("(c p) n -> p c n", p=P)
                for nch in range(2):
                    wst2 = stage.tile([P, D], F32, tag="wst2")
                    nc.sync.dma_start(wst2, wo3[:, nch, :])
                    nc.vector.tensor_copy(wo_bf[:, nch, :], wst2)

                def qk_piece(pool, tag_bufs, w_bf, b_sb, dest, nsub, sb):
                    # one [128, 512] slice of QT/KT: 8 accumulating matmuls
                    ps = pool.tile([P, 512], F32, tag=tag_bufs[0], bufs=tag_bufs[1], name="ps_qk")
                    for dc in range(DC):
                        nc.tensor.matmul(
                            ps,
                            lhsT=w_bf[:, dc, nsub * P : (nsub + 1) * P],
                            rhs=xT[:, dc, sb * 512 : (sb + 1) * 512],
                            start=(dc == 0),
                            stop=(dc == DC - 1),
                        )
                    nc.vector.tensor_scalar(
                        dest[:, nsub, sb * 512 : (sb + 1) * 512],
                        ps,
                        b_sb[:, nsub : nsub + 1],
                        None,
                        ADD,
                    )

                # KT in full (every attention block reads all of it) and the
                # first q-block of QT; remaining QT slices become PE filler
                # inside the attention loop.
                for nsub in range(2):
                    for sb in range(S // 512):
                        qk_piece(psA, ("proj", 4), wk_bf, bk_sb, KT, nsub, sb)
                for nsub in range(2):
                    qk_piece(psA, ("proj", 4), wq_bf, bq_sb, QT, nsub, 0)

            # ---- phase B: attention (scores transposed [k, q]) ----
            # PE filler pieces keep the TensorE dense (HAM-warm) while the
            # Activation engine paces the exp pipeline.
            with tc.tile_pool(name="psB", bufs=1, space="PSUM") as psB:

                def v_piece(sc):
                    ps = psB.tile([P, 512], F32, tag="gen", bufs=2, name="ps_v")
                    psv = ps[:, :NL]
                    for dc in range(DC):
                        nc.tensor.matmul(
                            psv,
                            lhsT=xT[:, dc, sc * P : (sc + 1) * P],
                            rhs=wv_bf[:, dc, :],
                            start=(dc == 0),
                            stop=(dc == DC - 1),
                        )
                    nc.vector.tensor_tensor(
                        V[:, sc],
                        psv.rearrange("p (h d) -> p h d", h=HL),
                        bv_sb.rearrange("p (h d) -> p h d", h=HL),
                        ADD,
                    )

                def y_piece(qc, mb):
                    psy = psB.tile([P, 512], F32, tag="gen", bufs=2, name="ps_y")
                    for nch in range(2):
                        nc.tensor.matmul(
                            psy,
                            lhsT=outT[:, nch, qc * P : (qc + 1) * P],
                            rhs=wo_bf[:, nch, mb * 512 : (mb + 1) * 512],
                            start=(nch == 0),
                            stop=(nch == 1),
                        )
                    yt = small.tile([P, 512], F32, tag="yt")
                    nc.vector.tensor_tensor(yt, psy, bo_sb[:, mb * 512 : (mb + 1) * 512], ADD)
                    (nc.sync if mb == 0 else nc.gpsimd).dma_start(
                        y[qc * P : (qc + 1) * P, mb * 512 : (mb + 1) * 512], yt
                    )

                filler = []
                filler.extend((lambda sc=sc: v_piece(sc)) for sc in range(SC))
                for sb in range(1, S // 512):
                    filler.extend(
                        (lambda nsub=nsub, sb=sb: qk_piece(psB, ("gen", 2), wq_bf, bq_sb, QT, nsub, sb))
                        for nsub in range(2)
                    )
                filler.reverse()  # consume with pop() in push order

                for qb in range(QB):
                    for hp in range(2):  # head pairs (2hp, 2hp+1)
                        n_pops = -(-SC // (KC // 2)) if (qb == 0 and hp == 0) else 1
                        hA, hB = 2 * hp, 2 * hp + 1
                        expA = expp.tile([P, KC, QBS], BF16, tag="exp")
                        expB = expp.tile([P, KC, QBS], BF16, tag="exp")
                        qA = QT[0:HEAD_DIM, hp, qb * QBS : (qb + 1) * QBS]
                        qB = QT[HEAD_DIM:P, hp, qb * QBS : (qb + 1) * QBS]
                        for g in range(KC // 2):
                            psa = psB.tile([P, 2, QBS], F32, tag="s", bufs=2)
                            psb = psB.tile([P, 2, QBS], F32, tag="s", bufs=2)
                            for j in range(2):
                                kc = 2 * g + j
                                # row-packed pair: head A on PE rows 0-63,
                                # head B on rows 64-127 (auto tile_position)
                                nc.tensor.matmul(
                                    psa[:, j],
                                    lhsT=KT[0:HEAD_DIM, hp, kc * P : (kc + 1) * P],
                                    rhs=qA,
                                    start=True,
                                    stop=True,
                                )
                                nc.tensor.matmul(
                                    psb[:, j],
                                    lhsT=KT[HEAD_DIM:P, hp, kc * P : (kc + 1) * P],
                                    rhs=qB,
                                    start=True,
                                    stop=True,
                                )
                            nc.scalar.activation(expA[:, 2 * g : 2 * g + 2], psa, EXP, scale=SCALE)
                            nc.scalar.activation(expB[:, 2 * g : 2 * g + 2], psb, EXP, scale=SCALE)
                            # Tile dependencies follow emission order, so all
                            # V pieces must be emitted before this block's PV
                            # loop: the first block drains two per group.
                            for _ in range(n_pops):
                                if filler:
                                    filler.pop()()

                        # PV col-packed (A cols 0-63, B cols 64-127) + rowsums
                        pv = psB.tile([P, QBS], F32, tag="pv", bufs=1)
                        sm = psB.tile([P, QBS], F32, tag="sum", bufs=1)
                        for kc in range(KC):
                            st, sp = (kc == 0), (kc == KC - 1)
                            nc.tensor.matmul(
                                pv[0:HEAD_DIM],
                                lhsT=V[:, kc, hA, :],
                                rhs=expA[:, kc, :],
                                start=st,
                                stop=sp,
                                skip_group_check=True,
                                tile_position=(0, 0),
                            )
                            nc.tensor.matmul(
                                pv[HEAD_DIM:P],
                                lhsT=V[:, kc, hB, :],
                                rhs=expB[:, kc, :],
                                start=st,
                                stop=sp,
                                skip_group_check=True,
                                tile_position=(0, 64),
                            )
                            # ones lhsT (M=64) replicates each head's rowsum
                            # across its 64 psum partitions - aligned for recip
                            nc.tensor.matmul(
                                sm[0:HEAD_DIM],
                                lhsT=ones,
                                rhs=expA[:, kc, :],
                                start=st,
                                stop=sp,
                                skip_group_check=True,
                                tile_position=(0, 0),
                            )
                            nc.tensor.matmul(
                                sm[HEAD_DIM:P],
                                lhsT=ones,
                                rhs=expB[:, kc, :],
                                start=st,
                                stop=sp,
                                skip_group_check=True,
                                tile_position=(0, 64),
                            )
                        # stage PV out of PSUM right away (frees the pv/sm
                        # slots for the next block); the slow DVE reciprocal
                        # and the normalize run off the critical path.
                        pvs = small.tile([P, QBS], F32, tag="pvs")
                        nc.vector.tensor_copy(pvs, pv)
                        rbc = small.tile([P, QBS], F32, tag="rbc")
                        nc.vector.reciprocal(rbc, sm)
                        nc.vector.tensor_tensor(
                            outT[:, hp, qb * QBS : (qb + 1) * QBS], pvs, rbc, MULT
                        )

                    # queue this q block's out-projection as filler
                    filler = [
                        (lambda qc=qc, mb=mb: y_piece(qc, mb))
                        for qc in range(qb * (QBS // P), (qb + 1) * (QBS // P))
                        for mb in range(2)
                    ][::-1] + filler

                # drain remaining filler (last block's y projection etc.)
                while filler:
                    filler.pop()()

    _split_excess_waits(nc)
    return nc


def shard_inputs(x, Wq, bq, Wk, bk, Wv, bv, Wo, bo):
    """Split full inputs into 8 per-core maps: core c -> (batch c//4, heads slice c%4)."""
    in_maps = []
    zeros_bo = np.zeros_like(bo)
    for c in range(8):
        b, g = c // 4, c % 4
        n0 = g * NL
        in_maps.append(
            {
                "x": np.ascontiguousarray(x[b]),
                "wq": np.ascontiguousarray(Wq[:, n0 : n0 + NL]),
                "wk": np.ascontiguousarray(Wk[:, n0 : n0 + NL]),
                "wv": np.ascontiguousarray(Wv[:, n0 : n0 + NL]),
                "bq": np.ascontiguousarray(bq[n0 : n0 + NL]),
                "bk": np.ascontiguousarray(bk[n0 : n0 + NL]),
                "bv": np.ascontiguousarray(bv[n0 : n0 + NL]),
                "wo": np.ascontiguousarray(Wo[n0 : n0 + NL, :]),
                "bo": bo if g == 0 else zeros_bo,
            }
        )
    return in_maps


_NC_CACHE = {}


def kernel(x, Wq, bq, Wk, bk, Wv, bv, Wo, bo, trace=False, tmpdir=None):
    from concourse.bass_utils import run_bass_kernel_spmd

    x = np.asarray(x, dtype=np.float32)
    args = [np.asarray(a, dtype=np.float32) for a in (Wq, bq, Wk, bk, Wv, bv, Wo, bo)]
    B, S, D = x.shape

    if S not in _NC_CACHE:
        _NC_CACHE[S] = build_bass(S)
    nc = _NC_CACHE[S]

    in_maps = shard_inputs(x, *args)
    res = run_bass_kernel_spmd(
        nc, in_maps, core_ids=list(range(8)), trace=trace, tmpdir=tmpdir
    )
    parts = [np.asarray(res.results[c]["y"]) for c in range(8)]
    out = np.empty((B, S, D), dtype=np.float32)
    for b in range(B):
        out[b] = parts[4 * b] + parts[4 * b + 1] + parts[4 * b + 2] + parts[4 * b + 3]
    if trace:
        kernel.last_result = res
    return out



# revision 6
# speedup vs baseline: 1.2028x; 1.2028x over previous
"""Trainium2 Bass kernel for nn_MultiHeadAttention (B=2, S=2048, D=1024, H=16).

Sharding: 8 cores = 2 (batch) x 4 (head groups of 4 heads / 256 proj dims).
Each core computes q/k/v projections for its 256-dim slice, attention for its
4 heads, and a partial out-projection y_part = attn_out @ Wo[slice].  The host
gather sums the 4 bf16 partials per batch in fp32 and adds bo.

v3 design (ACT-paced flat software pipeline):
 - x is pre-transposed AND pre-converted to bf16 on the host (xt [D, S]);
   weights host-converted to bf16.  No PE transposes, no on-chip casts.
 - The exp pipeline on the Activation engine is the hard floor (~16.8M exps
   per core; 128 x 1147ns ACTIVATEs = ~147us).  Everything is scheduled to
   keep ACT dense: one flat stream of 128 steps (8 blocks x 16 k-chunks);
   each step emits the 2 row-packed score matmuls for chunk kc -> one
   1024-elem ACTIVATE -> the col-packed PV matmuls lagged LAG steps behind,
   plus at most one woven projection/out-proj piece as PE filler.
 - Rowsum (softmax denominator) is NOT a PE matmul chain: exp chunks are
   accumulated elementwise on DVE (first chunks) + GpSimd (rest) in SBUF,
   with 4 accumulating ones-matmuls per block at the end.  This frees
   ~28us of PE time and a PSUM bank.
 - PSUM budget (8 banks): score tiles [128,2,512] x bufs=3 (6) + pv (1) +
   filler/rowsum (1).
 - DMA: sync(SP) ring is ~180GB/s, scalar ring ~93GB/s, gpsimd ring only
   ~24GB/s (software descriptors).  Bulk inputs go on sync+scalar; ALL y
   output writes go on sync.  gpsimd does no DMA.
"""

import sys

sys.path.insert(0, "/opt/trn_rl_repo")

import ml_dtypes
import numpy as np

import concourse.bass as bass
import concourse.mybir as mybir
import concourse.tile as _tile_mod
from concourse.tile import TileContext
from concourse.vector_clock import ScopedClock


def _drain_and_barrier_split_waits(self, tick_clock, wait_clock):
    """Replacement for TileContext._drain_and_barrier.

    The walrus build in this container only accepts one sync-wait command per
    CTRL instruction; the stock tail drain carries one wait per outstanding
    proc and fails codegen with "Too many sync wait commands".  Attach the
    waits to a nop first, then redistribute the surplus onto extra nops.
    """
    carrier = self.nc.sync.nop()
    wait_clock.add_sem_waits(carrier.ins, ScopedClock({None: tick_clock.global_clock}))
    si = carrier.ins.sync_info
    if si is not None and len(si.on_wait) > 1:
        waits = list(si.on_wait)
        carrier.ins.sync_info = mybir.SyncInfo(
            on_wait=[waits[0]], on_update=list(si.on_update)
        )
        for w in waits[1:]:
            extra = self.nc.sync.nop()
            extra.ins.sync_info = mybir.SyncInfo(on_wait=[w], on_update=[])
    self.nc.sync.drain()

    self.nc.all_engine_barrier()
    assert self.sems is not None
    popped = self.nc._tile_sem_poison_stack.pop()
    assert popped is self._sem_poison
    self.nc.clear_and_free_semaphores(list(self.sems.allocated().values()))
    self.nc.all_engine_barrier()


_tile_mod.TileContext._drain_and_barrier = _drain_and_barrier_split_waits


def _split_excess_waits(nc):
    """This container's walrus accepts only ONE sync-wait command per
    instruction.  Tile emits up to 3.  Hoist all but the last wait of each
    instruction onto fresh same-engine NoOps placed directly before it --
    sound because walrus lowers DMA waits into the issuing sequencer's
    pseudo-instruction, so waits always gate the same sequencer stream."""
    ctr = 0
    for fn in nc.m.functions:
        for blk in fn.blocks:
            rewritten = []
            changed = False
            for ins in blk.instructions:
                si = ins.sync_info
                if si is not None and len(si.on_wait) > 1:
                    waits = list(si.on_wait)
                    for w in waits[:-1]:
                        nop = mybir.InstNoOp(name=f"I-wsplit-{ctr}", ins=[], outs=[])
                        ctr += 1
                        nop.engine = ins.engine
                        nop.sync_info = mybir.SyncInfo(on_wait=[w], on_update=[])
                        nc.register_instruction(nop)
                        rewritten.append(nop)
                    ins.sync_info = mybir.SyncInfo(
                        on_wait=[waits[-1]], on_update=list(si.on_update)
                    )
                    changed = True
                rewritten.append(ins)
            if changed:
                blk.instructions = rewritten
    return nc


F32 = mybir.dt.float32
BF16 = mybir.dt.bfloat16
BF16_NP = ml_dtypes.bfloat16
ADD = mybir.AluOpType.add
MULT = mybir.AluOpType.mult
EXP = mybir.ActivationFunctionType.Exp

P = 128
D_MODEL = 1024
N_HEADS = 16
HEAD_DIM = 64
SCALE = HEAD_DIM**-0.5

NL = 256  # local projection dims (4 heads x 64)
HL = 4  # local heads
QBS = 512  # q block size for attention
LAG = 3  # steps PV trails the scores/exp pipeline


def build_bass(S: int) -> bass.Bass:
    """One SPMD program; every core runs it on its own shard."""
    D = D_MODEL
    DC = D // P  # 8
    SC = S // P  # 16
    KC = S // P  # 16
    QB = S // QBS  # 4

    nc = bass.Bass()
    xt = nc.declare_dram_parameter("xt", [D, S], BF16, isOutput=False)
    wq = nc.declare_dram_parameter("wq", [D, NL], BF16, isOutput=False)
    wk = nc.declare_dram_parameter("wk", [D, NL], BF16, isOutput=False)
    wv = nc.declare_dram_parameter("wv", [D, NL], BF16, isOutput=False)
    bq = nc.declare_dram_parameter("bq", [NL], F32, isOutput=False)
    bk = nc.declare_dram_parameter("bk", [NL], F32, isOutput=False)
    bv = nc.declare_dram_parameter("bv", [NL], F32, isOutput=False)
    wo = nc.declare_dram_parameter("wo", [NL, D], BF16, isOutput=False)
    y = nc.declare_dram_parameter("y", [S, D], BF16, isOutput=True)

    with TileContext(nc) as tc:
        with (
            tc.tile_pool(name="persist", bufs=1) as pp,
            tc.tile_pool(name="small", bufs=3) as small,
            tc.tile_pool(name="psum", bufs=1, space="PSUM") as psp,
        ):
            # ---- constants / ACT table warm-up ----
            ones = pp.tile([P, HEAD_DIM], BF16, name="ones")
            nc.vector.memset(ones, 1.0)
            warm_in = pp.tile([P, 1], F32, name="warm_in")
            nc.vector.memset(warm_in, 0.0)
            warm_out = pp.tile([P, 1], F32, name="warm_out")
            nc.scalar.activation(warm_out, warm_in, EXP)

            # ---- persistent activations ----
            xT = pp.tile([P, DC, S], BF16, name="xT")
            QT = pp.tile([P, 2, S], BF16, name="QT")
            KT = pp.tile([P, 2, S], BF16, name="KT")
            V = pp.tile([P, SC, HL, HEAD_DIM], BF16, name="V")
            outT = pp.tile([P, 2, S], BF16, name="outT")
            expT = pp.tile([P, KC, 2, QBS], BF16, name="expT")

            wq_sb = pp.tile([P, DC, NL], BF16, name="wq_sb")
            wk_sb = pp.tile([P, DC, NL], BF16, name="wk_sb")
            wv_sb = pp.tile([P, DC, NL], BF16, name="wv_sb")
            wo_sb = pp.tile([P, 2, D], BF16, name="wo_sb")
            bq_sb = pp.tile([P, 2], F32, name="bq_sb")
            bk_sb = pp.tile([P, 2], F32, name="bk_sb")
            bv_bc = pp.tile([P, NL], F32, name="bv_bc")

            xt3 = xt[:].rearrange("(c p) s -> p c s", p=P)

            # sync ring (fast): wk then x s-blocks 0,1,3; scalar ring: biases
            # first (tiny), wq, wv, x s-block 2, wo (needed latest).
            nc.sync.dma_start(wk_sb, wk[:].rearrange("(c p) n -> p c n", p=P))
            nc.sync.dma_start(xT[:, :, 0:512], xt3[:, :, 0:512])
            nc.sync.dma_start(xT[:, :, 512:1024], xt3[:, :, 512:1024])
            nc.sync.dma_start(xT[:, :, 1536:2048], xt3[:, :, 1536:2048])
            nc.scalar.dma_start(bq_sb, bq[:].rearrange("(o p) -> p o", p=P))
            nc.scalar.dma_start(bk_sb, bk[:].rearrange("(o p) -> p o", p=P))
            nc.scalar.dma_start(bv_bc, bv[:].unsqueeze(0).to_broadcast((P, NL)))
            nc.scalar.dma_start(wq_sb, wq[:].rearrange("(c p) n -> p c n", p=P))
            nc.scalar.dma_start(wv_sb, wv[:].rearrange("(c p) n -> p c n", p=P))
            nc.scalar.dma_start(xT[:, :, 1024:1536], xt3[:, :, 1024:1536])
            nc.scalar.dma_start(wo_sb, wo[:].rearrange("(c p) d -> p c d", p=P))

            # ---- projection / out-proj pieces (PE filler units) ----
            def proj_piece(w_sb, b_sb, dest, nsub, sb):
                ps = psp.tile([P, 512], F32, tag="gen", bufs=1, name="ps_p")
                for dc in range(DC):
                    nc.tensor.matmul(
                        ps,
                        lhsT=w_sb[:, dc, nsub * P : (nsub + 1) * P],
                        rhs=xT[:, dc, sb * 512 : (sb + 1) * 512],
                        start=(dc == 0),
                        stop=(dc == DC - 1),
                    )
                nc.vector.tensor_scalar(
                    dest[:, nsub, sb * 512 : (sb + 1) * 512],
                    ps,
                    b_sb[:, nsub : nsub + 1],
                    None,
                    ADD,
                )

            def v_piece(sc):
                ps = psp.tile([P, 512], F32, tag="gen", bufs=1, name="ps_v")
                psv = ps[:, :NL]
                for dc in range(DC):
                    nc.tensor.matmul(
                        psv,
                        lhsT=xT[:, dc, sc * P : (sc + 1) * P],
                        rhs=wv_sb[:, dc, :],
                        start=(dc == 0),
                        stop=(dc == DC - 1),
                    )
                nc.vector.tensor_tensor(
                    V[:, sc],
                    psv.rearrange("p (h d) -> p h d", h=HL),
                    bv_bc.rearrange("p (h d) -> p h d", h=HL),
                    ADD,
                )

            def y_piece(qc, mb):
                ps = psp.tile([P, 512], F32, tag="gen", bufs=1, name="ps_y")
                for nch in range(2):
                    nc.tensor.matmul(
                        ps,
                        lhsT=outT[:, nch, qc * P : (qc + 1) * P],
                        rhs=wo_sb[:, nch, mb * 512 : (mb + 1) * 512],
                        start=(nch == 0),
                        stop=(nch == 1),
                    )
                yt = small.tile([P, 512], BF16, tag="yt")
                nc.vector.tensor_copy(yt, ps)
                nc.sync.dma_start(
                    y[qc * P : (qc + 1) * P, mb * 512 : (mb + 1) * 512], yt
                )

            # ---- weave schedule: step index -> filler closures ----
            fill: dict[int, list] = {}

            def put(s, fn):
                fill.setdefault(s, []).append(fn)

            for kc in range(16):  # V just-in-time for block 0's PV (lag 3)
                put(kc + 1, lambda sc=kc: v_piece(sc))
            put(2, lambda: proj_piece(wk_sb, bk_sb, KT, 0, 1))
            put(6, lambda: proj_piece(wk_sb, bk_sb, KT, 0, 2))
            put(10, lambda: proj_piece(wk_sb, bk_sb, KT, 0, 3))
            put(12, lambda: proj_piece(wk_sb, bk_sb, KT, 1, 0))
            put(14, lambda: proj_piece(wq_sb, bq_sb, QT, 1, 0))
            put(17, lambda: proj_piece(wk_sb, bk_sb, KT, 1, 1))
            put(21, lambda: proj_piece(wk_sb, bk_sb, KT, 1, 2))
            put(25, lambda: proj_piece(wk_sb, bk_sb, KT, 1, 3))
            put(28, lambda: proj_piece(wq_sb, bq_sb, QT, 0, 1))
            put(30, lambda: proj_piece(wq_sb, bq_sb, QT, 1, 1))
            put(52, lambda: proj_piece(wq_sb, bq_sb, QT, 0, 2))
            put(56, lambda: proj_piece(wq_sb, bq_sb, QT, 1, 2))
            put(84, lambda: proj_piece(wq_sb, bq_sb, QT, 0, 3))
            put(88, lambda: proj_piece(wq_sb, bq_sb, QT, 1, 3))
            for j, base in ((0, 38), (1, 70), (2, 102)):  # y(qb j) pieces
                for k in range(8):
                    put(base + 2 * k, lambda qc=4 * j + k // 2, mb=k % 2: y_piece(qc, mb))

            # ---- flat 128-step stream ----
            blocks = [(qb, hp) for qb in range(QB) for hp in range(2)]
            steps = [(t, kc) for t in range(len(blocks)) for kc in range(KC)]
            pv_tiles: dict = {}
            acc_tiles: dict = {}

            def emit_pv(ls):
                lt, lkc = steps[ls]
                lqb, lhp = blocks[lt]
                if lt not in pv_tiles:
                    pv_tiles[lt] = psp.tile([P, QBS], F32, tag="pv", bufs=1, name="pv")
                pv = pv_tiles[lt]
                st, sp = (lkc == 0), (lkc == KC - 1)
                nc.tensor.matmul(
                    pv[0:HEAD_DIM],
                    lhsT=V[:, lkc, 2 * lhp, :],
                    rhs=expT[:, lkc, 0, :],
                    start=st,
                    stop=sp,
                    skip_group_check=True,
                    tile_position=(0, 0),
                )
                nc.tensor.matmul(
                    pv[HEAD_DIM:P],
                    lhsT=V[:, lkc, 2 * lhp + 1, :],
                    rhs=expT[:, lkc, 1, :],
                    start=st,
                    stop=sp,
                    skip_group_check=True,
                    tile_position=(0, 64),
                )
                if lkc == KC - 1:
                    finish_block(lt)

            def finish_block(t):
                qb, hp = blocks[t]
                accV, accG = acc_tiles.pop(t)
                # rowsum: accumulate the two SBUF accumulators via ones-matmuls
                smp = psp.tile([P, QBS], F32, tag="gen", bufs=1, name="smp")
                for j, acc in enumerate((accV, accG)):
                    st, sp = (j == 0), (j == 1)
                    nc.tensor.matmul(
                        smp[0:HEAD_DIM],
                        lhsT=ones,
                        rhs=acc[:, 0],
                        start=st,
                        stop=sp,
                        skip_group_check=True,
                        tile_position=(0, 0),
                    )
                    nc.tensor.matmul(
                        smp[HEAD_DIM:P],
                        lhsT=ones,
                        rhs=acc[:, 1],
                        start=st,
                        stop=sp,
                        skip_group_check=True,
                        tile_position=(0, 64),
                    )
                pv = pv_tiles.pop(t)
                pvs = small.tile([P, QBS], F32, tag="pvs")
                nc.vector.tensor_copy(pvs, pv)
                smsb = small.tile([P, QBS], F32, tag="smsb")
                nc.vector.tensor_copy(smsb, smp)
                rbc = small.tile([P, QBS], F32, tag="rbc")
                if t < len(blocks) - 1:
                    nc.vector.reciprocal(rbc, smsb)
                    nc.vector.tensor_tensor(
                        outT[:, hp, qb * QBS : (qb + 1) * QBS], pvs, rbc, MULT
                    )
                else:
                    # last block: normalize in halves so the final out-proj
                    # pieces can start as early as possible.
                    for h in range(2):
                        sl = slice(h * 256, (h + 1) * 256)
                        nc.vector.reciprocal(rbc[:, sl], smsb[:, sl])
                        nc.vector.tensor_tensor(
                            outT[:, hp, qb * QBS + h * 256 : qb * QBS + (h + 1) * 256],
                            pvs[:, sl],
                            rbc[:, sl],
                            MULT,
                        )
                        for qc in (4 * qb + 2 * h, 4 * qb + 2 * h + 1):
                            for mb in range(2):
                                y_piece(qc, mb)

            # phase A: first KT piece + first QT piece gate the stream
            proj_piece(wk_sb, bk_sb, KT, 0, 0)
            proj_piece(wq_sb, bq_sb, QT, 0, 0)

            for s, (t, kc) in enumerate(steps):
                qb, hp = blocks[t]
                for fn in fill.get(s, ()):
                    fn()
                ps = psp.tile([P, 2, QBS], F32, tag="s", bufs=3, name="ps_s")
                nc.tensor.matmul(
                    ps[:, 0],
                    lhsT=KT[0:HEAD_DIM, hp, kc * P : (kc + 1) * P],
                    rhs=QT[0:HEAD_DIM, hp, qb * QBS : (qb + 1) * QBS],
                    start=True,
                    stop=True,
                )
                nc.tensor.matmul(
                    ps[:, 1],
                    lhsT=KT[HEAD_DIM:P, hp, kc * P : (kc + 1) * P],
                    rhs=QT[HEAD_DIM:P, hp, qb * QBS : (qb + 1) * QBS],
                    start=True,
                    stop=True,
                )
                nc.scalar.activation(expT[:, kc], ps, EXP, scale=SCALE)
                # softmax-denominator accumulation: DVE takes the first
                # chunks, GpSimd the rest (block 0/1 lean on GpSimd since
                # DVE carries the V-piece bias adds there).
                nsplit = 4 if t < 2 else 8
                if kc == 0:
                    accV = small.tile([P, 2, QBS], BF16, tag="accV")
                    accG = small.tile([P, 2, QBS], BF16, tag="accG")
                    acc_tiles[t] = (accV, accG)
                    nc.vector.tensor_copy(accV, expT[:, kc])
                elif kc < nsplit:
                    accV = acc_tiles[t][0]
                    nc.vector.tensor_tensor(accV, accV, expT[:, kc], ADD)
                elif kc == nsplit:
                    accG = acc_tiles[t][1]
                    nc.gpsimd.tensor_copy(accG, expT[:, kc])
                else:
                    accG = acc_tiles[t][1]
                    nc.gpsimd.tensor_tensor(accG, accG, expT[:, kc], ADD)
                if s >= LAG:
                    emit_pv(s - LAG)
            for ls in range(len(steps) - LAG, len(steps)):
                emit_pv(ls)

    _split_excess_waits(nc)
    return nc


def shard_inputs(x, Wq, bq, Wk, bk, Wv, bv, Wo, bo):
    """Split full inputs into 8 per-core maps: core c -> (batch c//4, head
    group c%4).  x is transposed + bf16-converted per batch on the host."""
    B = x.shape[0]
    xts = [np.ascontiguousarray(x[b].T).astype(BF16_NP) for b in range(B)]
    in_maps = []
    for c in range(8):
        b, g = c // 4, c % 4
        n0 = g * NL
        in_maps.append(
            {
                "xt": xts[b],
                "wq": np.ascontiguousarray(Wq[:, n0 : n0 + NL]).astype(BF16_NP),
                "wk": np.ascontiguousarray(Wk[:, n0 : n0 + NL]).astype(BF16_NP),
                "wv": np.ascontiguousarray(Wv[:, n0 : n0 + NL]).astype(BF16_NP),
                "bq": np.ascontiguousarray(bq[n0 : n0 + NL]),
                "bk": np.ascontiguousarray(bk[n0 : n0 + NL]),
                "bv": np.ascontiguousarray(bv[n0 : n0 + NL]),
                "wo": np.ascontiguousarray(Wo[n0 : n0 + NL, :]).astype(BF16_NP),
            }
        )
    return in_maps


_NC_CACHE = {}


def kernel(x, Wq, bq, Wk, bk, Wv, bv, Wo, bo, trace=False, tmpdir=None):
    from concourse.bass_utils import run_bass_kernel_spmd

    x = np.asarray(x, dtype=np.float32)
    args = [np.asarray(a, dtype=np.float32) for a in (Wq, bq, Wk, bk, Wv, bv, Wo, bo)]
    B, S, D = x.shape

    if S not in _NC_CACHE:
        _NC_CACHE[S] = build_bass(S)
    nc = _NC_CACHE[S]

    in_maps = shard_inputs(x, *args)
    res = run_bass_kernel_spmd(
        nc, in_maps, core_ids=list(range(8)), trace=trace, tmpdir=tmpdir
    )
    if trace:
        kernel.last_result = res
    bo_f = args[7]
    parts = [np.asarray(res.results[c]["y"]).astype(np.float32) for c in range(8)]
    out = np.empty((B, S, D), dtype=np.float32)
    for b in range(B):
        out[b] = parts[4 * b] + parts[4 * b + 1] + parts[4 * b + 2] + parts[4 * b + 3]
        out[b] += bo_f
    return out


# revision 17
# speedup vs baseline: 1.6708x; 1.3891x over previous
"""Trainium2 Bass kernel for nn_MultiHeadAttention (B=2, S=2048, D=1024, H=16).

Sharding: 8 cores = 2 (batch) x 4 (head groups of 4 heads / 256 proj dims).
Each core computes q/k/v projections for its 256-dim slice, attention for its
4 heads, and a partial out-projection y_part = attn_out @ Wo[slice].  The host
gather sums the 4 bf16 partials per batch in fp32 and adds bo.

v4 design (ACT-paced flat software pipeline):
 - x is pre-transposed, bf16-converted AND prearranged to the SBUF layout on
   the host; weights likewise.  No PE transposes, no on-chip casts, and every
   input DMA is 128 descriptors of >=2KB contiguous per partition.
 - The exp pipeline on the Activation engine is the hard floor (~16.8M exps
   per core; 128 x 1147ns ACTIVATEs = ~147us).  Everything is scheduled to
   keep ACT dense: one flat stream of 128 steps (8 blocks x 16 k-chunks);
   each step emits the 2 row-packed score matmuls for chunk kc -> one
   1024-elem ACTIVATE -> the col-packed PV + rowsum matmuls lagged LAG steps
   behind, plus at most one woven projection/out-proj piece as PE filler.
 - Rowsums via ones-matmuls (M=64 -> replicated across each head's partition
   range, aligned for the reciprocal).  (An SBUF-accumulator offload to
   DVE/GpSimd was tried and measured SLOWER: gpsimd tensor_tensor is
   ~2.1us/op and the serial chain head-of-line blocked every block.)
 - PSUM budget (8 banks): score tiles [128,2,512] x bufs=2 (4) + pv (1) +
   sm (1) + filler (2).
 - DMA: sync(SP) ring is ~180GB/s, scalar ring ~93GB/s, gpsimd ring only
   ~24GB/s (software descriptors).  Bulk inputs go on sync+scalar; ALL y
   output writes go on sync.  gpsimd does no DMA.
"""

import sys

sys.path.insert(0, "/opt/trn_rl_repo")

import ml_dtypes
import numpy as np

import concourse.bass as bass
import concourse.mybir as mybir
import concourse.tile as _tile_mod
from concourse.tile import TileContext
from concourse.vector_clock import ScopedClock


def _drain_and_barrier_split_waits(self, tick_clock, wait_clock):
    """Replacement for TileContext._drain_and_barrier.

    The walrus build in this container only accepts one sync-wait command per
    CTRL instruction; the stock tail drain carries one wait per outstanding
    proc and fails codegen with "Too many sync wait commands".  Attach the
    waits to a nop first, then redistribute the surplus onto extra nops.
    """
    carrier = self.nc.sync.nop()
    wait_clock.add_sem_waits(carrier.ins, ScopedClock({None: tick_clock.global_clock}))
    si = carrier.ins.sync_info
    if si is not None and len(si.on_wait) > 1:
        waits = list(si.on_wait)
        carrier.ins.sync_info = mybir.SyncInfo(
            on_wait=[waits[0]], on_update=list(si.on_update)
        )
        for w in waits[1:]:
            extra = self.nc.sync.nop()
            extra.ins.sync_info = mybir.SyncInfo(on_wait=[w], on_update=[])
    self.nc.sync.drain()

    self.nc.all_engine_barrier()
    assert self.sems is not None
    popped = self.nc._tile_sem_poison_stack.pop()
    assert popped is self._sem_poison
    self.nc.clear_and_free_semaphores(list(self.sems.allocated().values()))
    self.nc.all_engine_barrier()


_tile_mod.TileContext._drain_and_barrier = _drain_and_barrier_split_waits


def _split_excess_waits(nc):
    """This container's walrus accepts only ONE sync-wait command per
    instruction.  Tile emits up to 3.  Hoist all but the last wait of each
    instruction onto fresh same-engine NoOps placed directly before it --
    sound because walrus lowers DMA waits into the issuing sequencer's
    pseudo-instruction, so waits always gate the same sequencer stream."""
    ctr = 0
    for fn in nc.m.functions:
        for blk in fn.blocks:
            rewritten = []
            changed = False
            for ins in blk.instructions:
                si = ins.sync_info
                if si is not None and len(si.on_wait) > 1:
                    waits = list(si.on_wait)
                    for w in waits[:-1]:
                        nop = mybir.InstNoOp(name=f"I-wsplit-{ctr}", ins=[], outs=[])
                        ctr += 1
                        nop.engine = ins.engine
                        nop.sync_info = mybir.SyncInfo(on_wait=[w], on_update=[])
                        nc.register_instruction(nop)
                        rewritten.append(nop)
                    ins.sync_info = mybir.SyncInfo(
                        on_wait=[waits[-1]], on_update=list(si.on_update)
                    )
                    changed = True
                rewritten.append(ins)
            if changed:
                blk.instructions = rewritten
    return nc


F32 = mybir.dt.float32
BF16 = mybir.dt.bfloat16
BF16_NP = ml_dtypes.bfloat16
ADD = mybir.AluOpType.add
MULT = mybir.AluOpType.mult
EXP = mybir.ActivationFunctionType.Exp

P = 128
D_MODEL = 1024
N_HEADS = 16
HEAD_DIM = 64
SCALE = HEAD_DIM**-0.5

NL = 256  # local projection dims (4 heads x 64)
HL = 4  # local heads
QBS = 512  # q block size for attention
LAG = 3  # steps PV trails the scores/exp pipeline


def build_bass(S: int) -> bass.Bass:
    """One SPMD program; every core runs it on its own shard."""
    D = D_MODEL
    DC = D // P  # 8
    SC = S // P  # 16
    KC = S // P  # 16
    QB = S // QBS  # 4

    nc = bass.Bass()
    # all inputs host-prearranged to partition-major SBUF layouts so every
    # DMA is 128 descriptors of >=2KB contiguous per partition.
    xt = nc.declare_dram_parameter("xt", [P, QB, DC, 512], BF16, isOutput=False)
    wq = nc.declare_dram_parameter("wq", [P, DC, NL], BF16, isOutput=False)
    wk = nc.declare_dram_parameter("wk", [P, DC, NL], BF16, isOutput=False)
    wv = nc.declare_dram_parameter("wv", [P, DC, NL], BF16, isOutput=False)
    bq = nc.declare_dram_parameter("bq", [NL], F32, isOutput=False)
    bk = nc.declare_dram_parameter("bk", [NL], F32, isOutput=False)
    bv = nc.declare_dram_parameter("bv", [NL], F32, isOutput=False)
    wo = nc.declare_dram_parameter("wo", [P, 2, D], BF16, isOutput=False)
    y = nc.declare_dram_parameter("y", [S, D], BF16, isOutput=True)

    with TileContext(nc) as tc:
        with (
            tc.tile_pool(name="persist", bufs=1) as pp,
            tc.tile_pool(name="small", bufs=3) as small,
            tc.tile_pool(name="psum", bufs=1, space="PSUM") as psp,
        ):
            # ---- constants / ACT table warm-up ----
            ones = pp.tile([P, HEAD_DIM], BF16, name="ones")
            nc.vector.memset(ones, 1.0)
            warm_in = pp.tile([P, 1], F32, name="warm_in")
            nc.vector.memset(warm_in, 0.0)
            warm_out = pp.tile([P, 1], F32, name="warm_out")
            nc.scalar.activation(warm_out, warm_in, EXP)

            # ---- persistent activations ----
            xT = pp.tile([P, QB, DC, 512], BF16, name="xT")  # s-block major
            QT = pp.tile([P, 2, S], BF16, name="QT")
            KT = pp.tile([P, 2, S], BF16, name="KT")
            V = pp.tile([P, SC, HL, HEAD_DIM], BF16, name="V")
            outT = pp.tile([P, 2, S], BF16, name="outT")
            expT = pp.tile([P, KC, 2, QBS], BF16, name="expT")

            wq_sb = pp.tile([P, DC, NL], BF16, name="wq_sb")
            wk_sb = pp.tile([P, DC, NL], BF16, name="wk_sb")
            wv_sb = pp.tile([P, DC, NL], BF16, name="wv_sb")
            wo_sb = pp.tile([P, 2, D], BF16, name="wo_sb")
            bq_sb = pp.tile([P, 2], F32, name="bq_sb")
            bk_sb = pp.tile([P, 2], F32, name="bk_sb")
            bv_bc = pp.tile([P, NL], F32, name="bv_bc")

            # sync ring (fast): wk, biases (tiny, needed by first projection
            # bias-adds), then x s-blocks 0,1,3; scalar ring: wq, wv, x
            # s-block 2, wo (needed latest).
            nc.sync.dma_start(wk_sb, wk[:])
            nc.sync.dma_start(bq_sb, bq[:].rearrange("(o p) -> p o", p=P))
            nc.sync.dma_start(bk_sb, bk[:].rearrange("(o p) -> p o", p=P))
            nc.sync.dma_start(bv_bc, bv[:].unsqueeze(0).to_broadcast((P, NL)))
            nc.sync.dma_start(xT[:, 0], xt[:, 0])
            nc.sync.dma_start(xT[:, 1], xt[:, 1])
            nc.sync.dma_start(xT[:, 3], xt[:, 3])
            nc.scalar.dma_start(wq_sb, wq[:])
            nc.scalar.dma_start(wv_sb, wv[:])
            nc.scalar.dma_start(xT[:, 2], xt[:, 2])
            nc.scalar.dma_start(wo_sb, wo[:])

            # ---- projection / out-proj pieces (PE filler units) ----
            def proj_piece(w_sb, b_sb, dest, nsub, sb):
                ps = psp.tile([P, 512], F32, tag="gen", bufs=2, name="ps_p")
                for dc in range(DC):
                    nc.tensor.matmul(
                        ps,
                        lhsT=w_sb[:, dc, nsub * P : (nsub + 1) * P],
                        rhs=xT[:, sb, dc, :],
                        start=(dc == 0),
                        stop=(dc == DC - 1),
                    )
                nc.vector.tensor_scalar(
                    dest[:, nsub, sb * 512 : (sb + 1) * 512],
                    ps,
                    b_sb[:, nsub : nsub + 1],
                    None,
                    ADD,
                )

            def v_piece(sc):
                ps = psp.tile([P, 512], F32, tag="gen", bufs=2, name="ps_v")
                psv = ps[:, :NL]
                for dc in range(DC):
                    nc.tensor.matmul(
                        psv,
                        lhsT=xT[:, sc // 4, dc, (sc % 4) * P : (sc % 4 + 1) * P],
                        rhs=wv_sb[:, dc, :],
                        start=(dc == 0),
                        stop=(dc == DC - 1),
                    )
                nc.vector.tensor_tensor(
                    V[:, sc],
                    psv.rearrange("p (h d) -> p h d", h=HL),
                    bv_bc.rearrange("p (h d) -> p h d", h=HL),
                    ADD,
                )

            def y_piece(qc, mb):
                ps = psp.tile([P, 512], F32, tag="gen", bufs=2, name="ps_y")
                for nch in range(2):
                    nc.tensor.matmul(
                        ps,
                        lhsT=outT[:, nch, qc * P : (qc + 1) * P],
                        rhs=wo_sb[:, nch, mb * 512 : (mb + 1) * 512],
                        start=(nch == 0),
                        stop=(nch == 1),
                    )
                yt = small.tile([P, 512], BF16, tag="yt")
                nc.vector.tensor_copy(yt, ps)
                nc.sync.dma_start(
                    y[qc * P : (qc + 1) * P, mb * 512 : (mb + 1) * 512], yt
                )

            # ---- weave schedule: step index -> filler closures ----
            fill: dict[int, list] = {}

            def put(s, fn):
                fill.setdefault(s, []).append(fn)

            for kc in range(16):  # V just-in-time for block 0's PV (lag 3)
                put(kc + 1, lambda sc=kc: v_piece(sc))
            put(2, lambda: proj_piece(wk_sb, bk_sb, KT, 0, 1))
            put(6, lambda: proj_piece(wk_sb, bk_sb, KT, 0, 2))
            put(10, lambda: proj_piece(wk_sb, bk_sb, KT, 0, 3))
            put(12, lambda: proj_piece(wk_sb, bk_sb, KT, 1, 0))
            put(14, lambda: proj_piece(wq_sb, bq_sb, QT, 1, 0))
            put(17, lambda: proj_piece(wk_sb, bk_sb, KT, 1, 1))
            put(21, lambda: proj_piece(wk_sb, bk_sb, KT, 1, 2))
            put(25, lambda: proj_piece(wk_sb, bk_sb, KT, 1, 3))
            put(28, lambda: proj_piece(wq_sb, bq_sb, QT, 0, 1))
            put(30, lambda: proj_piece(wq_sb, bq_sb, QT, 1, 1))
            put(52, lambda: proj_piece(wq_sb, bq_sb, QT, 0, 2))
            put(56, lambda: proj_piece(wq_sb, bq_sb, QT, 1, 2))
            put(84, lambda: proj_piece(wq_sb, bq_sb, QT, 0, 3))
            put(88, lambda: proj_piece(wq_sb, bq_sb, QT, 1, 3))
            for j, base in ((0, 38), (1, 70), (2, 102)):  # y(qb j) pieces
                for k in range(8):
                    put(base + 2 * k, lambda qc=4 * j + k // 2, mb=k % 2: y_piece(qc, mb))

            # ---- flat 128-step stream ----
            blocks = [(qb, hp) for qb in range(QB) for hp in range(2)]
            steps = [(t, kc) for t in range(len(blocks)) for kc in range(KC)]
            pv_tiles: dict = {}

            def emit_pv(ls):
                lt, lkc = steps[ls]
                lqb, lhp = blocks[lt]
                if lt not in pv_tiles:
                    pv_tiles[lt] = (
                        psp.tile([P, QBS], F32, tag="pv", bufs=1, name="pv"),
                        psp.tile([P, QBS], F32, tag="sum", bufs=1, name="sm"),
                    )
                pv, sm = pv_tiles[lt]
                st, sp = (lkc == 0), (lkc == KC - 1)
                nc.tensor.matmul(
                    pv[0:HEAD_DIM],
                    lhsT=V[:, lkc, 2 * lhp, :],
                    rhs=expT[:, lkc, 0, :],
                    start=st,
                    stop=sp,
                    skip_group_check=True,
                    tile_position=(0, 0),
                )
                nc.tensor.matmul(
                    pv[HEAD_DIM:P],
                    lhsT=V[:, lkc, 2 * lhp + 1, :],
                    rhs=expT[:, lkc, 1, :],
                    start=st,
                    stop=sp,
                    skip_group_check=True,
                    tile_position=(0, 64),
                )
                nc.tensor.matmul(
                    sm[0:HEAD_DIM],
                    lhsT=ones,
                    rhs=expT[:, lkc, 0, :],
                    start=st,
                    stop=sp,
                    skip_group_check=True,
                    tile_position=(0, 0),
                )
                nc.tensor.matmul(
                    sm[HEAD_DIM:P],
                    lhsT=ones,
                    rhs=expT[:, lkc, 1, :],
                    start=st,
                    stop=sp,
                    skip_group_check=True,
                    tile_position=(0, 64),
                )
                if lkc == KC - 1:
                    finish_block(lt)

            def finish_block(t):
                qb, hp = blocks[t]
                pv, sm = pv_tiles.pop(t)
                pvs = small.tile([P, QBS], F32, tag="pvs")
                nc.vector.tensor_copy(pvs, pv)
                smsb = small.tile([P, QBS], F32, tag="smsb")
                nc.vector.tensor_copy(smsb, sm)
                rbc = small.tile([P, QBS], F32, tag="rbc")
                if t < len(blocks) - 1:
                    nc.vector.reciprocal(rbc, smsb)
                    nc.vector.tensor_tensor(
                        outT[:, hp, qb * QBS : (qb + 1) * QBS], pvs, rbc, MULT
                    )
                else:
                    # last block: normalize in halves so the final out-proj
                    # pieces can start as early as possible.
                    for h in range(2):
                        sl = slice(h * 256, (h + 1) * 256)
                        nc.vector.reciprocal(rbc[:, sl], smsb[:, sl])
                        nc.vector.tensor_tensor(
                            outT[:, hp, qb * QBS + h * 256 : qb * QBS + (h + 1) * 256],
                            pvs[:, sl],
                            rbc[:, sl],
                            MULT,
                        )
                        for qc in (4 * qb + 2 * h, 4 * qb + 2 * h + 1):
                            for mb in range(2):
                                y_piece(qc, mb)

            # phase A: first KT piece + first QT piece gate the stream
            proj_piece(wk_sb, bk_sb, KT, 0, 0)
            proj_piece(wq_sb, bq_sb, QT, 0, 0)

            for s, (t, kc) in enumerate(steps):
                qb, hp = blocks[t]
                for fn in fill.get(s, ()):
                    fn()
                ps = psp.tile([P, 2, QBS], F32, tag="s", bufs=2, name="ps_s")
                nc.tensor.matmul(
                    ps[:, 0],
                    lhsT=KT[0:HEAD_DIM, hp, kc * P : (kc + 1) * P],
                    rhs=QT[0:HEAD_DIM, hp, qb * QBS : (qb + 1) * QBS],
                    start=True,
                    stop=True,
                )
                nc.tensor.matmul(
                    ps[:, 1],
                    lhsT=KT[HEAD_DIM:P, hp, kc * P : (kc + 1) * P],
                    rhs=QT[HEAD_DIM:P, hp, qb * QBS : (qb + 1) * QBS],
                    start=True,
                    stop=True,
                )
                nc.scalar.activation(expT[:, kc], ps, EXP, scale=SCALE)
                if s >= LAG:
                    emit_pv(s - LAG)
            for ls in range(len(steps) - LAG, len(steps)):
                emit_pv(ls)

    _split_excess_waits(nc)
    return nc


def _w_pmajor(W):
    """[D, NL] -> [128, DC, NL] partition-major (p = d % 128, dc = d // 128)."""
    D, n = W.shape
    return np.ascontiguousarray(
        W.reshape(D // 128, 128, n).transpose(1, 0, 2)
    ).astype(BF16_NP)


def shard_inputs(x, Wq, bq, Wk, bk, Wv, bv, Wo, bo):
    """Split full inputs into 8 per-core maps: core c -> (batch c//4, head
    group c%4).  x is transposed, bf16-converted, AND prearranged to the
    SBUF layout [p, s_block, dc, s'] on the host so device DMAs are 128
    descriptors of 8KB contiguous."""
    B, S, D = x.shape
    xts = []
    for b in range(B):
        # x[b] [S, D] -> xT [D, S] -> [dc, p, sb, s'] -> [p, sb, dc, s']
        xt = x[b].T.reshape(D // 128, 128, S // 512, 512).transpose(1, 2, 0, 3)
        xts.append(np.ascontiguousarray(xt).astype(BF16_NP))
    in_maps = []
    for c in range(8):
        b, g = c // 4, c % 4
        n0 = g * NL
        in_maps.append(
            {
                "xt": xts[b],
                "wq": _w_pmajor(Wq[:, n0 : n0 + NL]),
                "wk": _w_pmajor(Wk[:, n0 : n0 + NL]),
                "wv": _w_pmajor(Wv[:, n0 : n0 + NL]),
                "bq": np.ascontiguousarray(bq[n0 : n0 + NL]),
                "bk": np.ascontiguousarray(bk[n0 : n0 + NL]),
                "bv": np.ascontiguousarray(bv[n0 : n0 + NL]),
                "wo": _w_pmajor(Wo[n0 : n0 + NL, :]),
            }
        )
    return in_maps


_NC_CACHE = {}


def kernel(x, Wq, bq, Wk, bk, Wv, bv, Wo, bo, trace=False, tmpdir=None):
    from concourse.bass_utils import run_bass_kernel_spmd

    x = np.asarray(x, dtype=np.float32)
    args = [np.asarray(a, dtype=np.float32) for a in (Wq, bq, Wk, bk, Wv, bv, Wo, bo)]
    B, S, D = x.shape

    if S not in _NC_CACHE:
        _NC_CACHE[S] = build_bass(S)
    nc = _NC_CACHE[S]

    in_maps = shard_inputs(x, *args)
    res = run_bass_kernel_spmd(
        nc, in_maps, core_ids=list(range(8)), trace=trace, tmpdir=tmpdir
    )
    if trace:
        kernel.last_result = res
    bo_f = args[7]
    parts = [np.asarray(res.results[c]["y"]).astype(np.float32) for c in range(8)]
    out = np.empty((B, S, D), dtype=np.float32)
    for b in range(B):
        out[b] = parts[4 * b] + parts[4 * b + 1] + parts[4 * b + 2] + parts[4 * b + 3]
        out[b] += bo_f
    return out


# revision 23
# speedup vs baseline: 1.7296x; 1.0352x over previous
"""Trainium2 Bass kernel for nn_MultiHeadAttention (B=2, S=2048, D=1024, H=16).

Sharding: 8 cores = 2 (batch) x 4 (head groups of 4 heads / 256 proj dims).
Each core computes q/k/v projections for its 256-dim slice, attention for its
4 heads, and a partial out-projection y_part = attn_out @ Wo[slice].  The host
gather sums the 4 bf16 partials per batch in fp32 and adds bo.

v4 design (ACT-paced flat software pipeline):
 - x is pre-transposed, bf16-converted AND prearranged to the SBUF layout on
   the host; weights likewise.  No PE transposes, no on-chip casts, and every
   input DMA is 128 descriptors of >=2KB contiguous per partition.
 - The exp pipeline on the Activation engine is the hard floor (~16.8M exps
   per core; 128 x 1147ns ACTIVATEs = ~147us).  Everything is scheduled to
   keep ACT dense: one flat stream of 128 steps (8 blocks x 16 k-chunks);
   each step emits the 2 row-packed score matmuls for chunk kc -> one
   1024-elem ACTIVATE -> the col-packed PV + rowsum matmuls lagged LAG steps
   behind, plus at most one woven projection/out-proj piece as PE filler.
 - Rowsums via ones-matmuls (M=64 -> replicated across each head's partition
   range, aligned for the reciprocal).  (An SBUF-accumulator offload to
   DVE/GpSimd was tried and measured SLOWER: gpsimd tensor_tensor is
   ~2.1us/op and the serial chain head-of-line blocked every block.)
 - PSUM budget (8 banks): score tiles [128,2,512] x bufs=2 (4) + pv (1) +
   sm (1) + filler (2).
 - DMA: sync(SP) ring is ~180GB/s, scalar ring ~93GB/s, gpsimd ring only
   ~24GB/s (software descriptors).  Bulk inputs go on sync+scalar; ALL y
   output writes go on sync.  gpsimd does no DMA.
"""

import sys

sys.path.insert(0, "/opt/trn_rl_repo")

import ml_dtypes
import numpy as np

import concourse.bass as bass
import concourse.mybir as mybir
import concourse.tile as _tile_mod
from concourse.tile import TileContext
from concourse.vector_clock import ScopedClock


def _drain_and_barrier_split_waits(self, tick_clock, wait_clock):
    """Replacement for TileContext._drain_and_barrier.

    The walrus build in this container only accepts one sync-wait command per
    CTRL instruction; the stock tail drain carries one wait per outstanding
    proc and fails codegen with "Too many sync wait commands".  Attach the
    waits to a nop first, then redistribute the surplus onto extra nops.
    """
    carrier = self.nc.sync.nop()
    wait_clock.add_sem_waits(carrier.ins, ScopedClock({None: tick_clock.global_clock}))
    si = carrier.ins.sync_info
    if si is not None and len(si.on_wait) > 1:
        waits = list(si.on_wait)
        carrier.ins.sync_info = mybir.SyncInfo(
            on_wait=[waits[0]], on_update=list(si.on_update)
        )
        for w in waits[1:]:
            extra = self.nc.sync.nop()
            extra.ins.sync_info = mybir.SyncInfo(on_wait=[w], on_update=[])
    self.nc.sync.drain()

    self.nc.all_engine_barrier()
    assert self.sems is not None
    popped = self.nc._tile_sem_poison_stack.pop()
    assert popped is self._sem_poison
    self.nc.clear_and_free_semaphores(list(self.sems.allocated().values()))
    self.nc.all_engine_barrier()


_tile_mod.TileContext._drain_and_barrier = _drain_and_barrier_split_waits


def _split_excess_waits(nc):
    """This container's walrus accepts only ONE sync-wait command per
    instruction.  Tile emits up to 3.  Hoist all but the last wait of each
    instruction onto fresh same-engine NoOps placed directly before it --
    sound because walrus lowers DMA waits into the issuing sequencer's
    pseudo-instruction, so waits always gate the same sequencer stream."""
    ctr = 0
    for fn in nc.m.functions:
        for blk in fn.blocks:
            rewritten = []
            changed = False
            for ins in blk.instructions:
                si = ins.sync_info
                if si is not None and len(si.on_wait) > 1:
                    waits = list(si.on_wait)
                    for w in waits[:-1]:
                        nop = mybir.InstNoOp(name=f"I-wsplit-{ctr}", ins=[], outs=[])
                        ctr += 1
                        nop.engine = ins.engine
                        nop.sync_info = mybir.SyncInfo(on_wait=[w], on_update=[])
                        nc.register_instruction(nop)
                        rewritten.append(nop)
                    ins.sync_info = mybir.SyncInfo(
                        on_wait=[waits[-1]], on_update=list(si.on_update)
                    )
                    changed = True
                rewritten.append(ins)
            if changed:
                blk.instructions = rewritten
    return nc


F32 = mybir.dt.float32
BF16 = mybir.dt.bfloat16
BF16_NP = ml_dtypes.bfloat16
ADD = mybir.AluOpType.add
MULT = mybir.AluOpType.mult
EXP = mybir.ActivationFunctionType.Exp

P = 128
D_MODEL = 1024
N_HEADS = 16
HEAD_DIM = 64
SCALE = HEAD_DIM**-0.5

NL = 256  # local projection dims (4 heads x 64)
HL = 4  # local heads
QBS = 512  # q block size for attention
LAG = 3  # steps PV trails the scores/exp pipeline


def build_bass(S: int) -> bass.Bass:
    """One SPMD program; every core runs it on its own shard."""
    D = D_MODEL
    DC = D // P  # 8
    SC = S // P  # 16
    KC = S // P  # 16
    QB = S // QBS  # 4

    nc = bass.Bass()
    # all inputs host-prearranged to partition-major SBUF layouts so every
    # DMA is 128 descriptors of >=2KB contiguous per partition.
    xt = nc.declare_dram_parameter("xt", [P, QB, DC, 512], BF16, isOutput=False)
    wq = nc.declare_dram_parameter("wq", [P, DC, NL], BF16, isOutput=False)
    wk = nc.declare_dram_parameter("wk", [P, DC, NL], BF16, isOutput=False)
    wv = nc.declare_dram_parameter("wv", [P, DC, NL], BF16, isOutput=False)
    bq = nc.declare_dram_parameter("bq", [NL], F32, isOutput=False)
    bk = nc.declare_dram_parameter("bk", [NL], F32, isOutput=False)
    bv = nc.declare_dram_parameter("bv", [NL], F32, isOutput=False)
    wo = nc.declare_dram_parameter("wo", [P, 2, D], BF16, isOutput=False)
    y = nc.declare_dram_parameter("y", [S, D], BF16, isOutput=True)

    with TileContext(nc) as tc:
        with (
            tc.tile_pool(name="persist", bufs=1) as pp,
            tc.tile_pool(name="small", bufs=3) as small,
            tc.tile_pool(name="psum", bufs=1, space="PSUM") as psp,
        ):
            # ---- constants / ACT table warm-up ----
            ones = pp.tile([P, HEAD_DIM], BF16, name="ones")
            nc.vector.memset(ones, 1.0)
            warm_in = pp.tile([P, 1], F32, name="warm_in")
            nc.vector.memset(warm_in, 0.0)
            warm_out = pp.tile([P, 1], F32, name="warm_out")
            nc.scalar.activation(warm_out, warm_in, EXP)
            # HAM warm-up: dummy matmuls during the input-DMA wait so the
            # first real projection pieces run at the full 2.4GHz clock.
            junk = pp.tile([P, 512], BF16, name="junk")
            nc.vector.memset(junk, 0.0)
            ps_w = psp.tile([P, 512], F32, tag="gen", bufs=2, name="ps_warm")
            for i in range(20):
                nc.tensor.matmul(
                    ps_w[0:HEAD_DIM], lhsT=ones, rhs=junk,
                    start=(i == 0), stop=(i == 19),
                )

            # ---- persistent activations ----
            xT = pp.tile([P, QB, DC, 512], BF16, name="xT")  # s-block major
            QT = pp.tile([P, 2, S], BF16, name="QT")
            KT = pp.tile([P, 2, S], BF16, name="KT")
            V = pp.tile([P, SC, HL, HEAD_DIM], BF16, name="V")
            outT = pp.tile([P, 2, S], BF16, name="outT")
            expT = pp.tile([P, KC, 2, QBS], BF16, name="expT")

            wq_sb = pp.tile([P, DC, NL], BF16, name="wq_sb")
            wk_sb = pp.tile([P, DC, NL], BF16, name="wk_sb")
            wv_sb = pp.tile([P, DC, NL], BF16, name="wv_sb")
            wo_sb = pp.tile([P, 2, D], BF16, name="wo_sb")
            bq_sb = pp.tile([P, 2], F32, name="bq_sb")
            bk_sb = pp.tile([P, 2], F32, name="bk_sb")
            bv_bc = pp.tile([P, NL], F32, name="bv_bc")

            # x s-block 0 split across BOTH rings so the first projection
            # pieces unblock earliest; sync ring (fast) carries the rest of x.
            nc.sync.dma_start(xT[:, 0, 0:4], xt[:, 0, 0:4])
            nc.sync.dma_start(wk_sb, wk[:])
            nc.sync.dma_start(bq_sb, bq[:].rearrange("(o p) -> p o", p=P))
            nc.sync.dma_start(bk_sb, bk[:].rearrange("(o p) -> p o", p=P))
            nc.sync.dma_start(bv_bc, bv[:].unsqueeze(0).to_broadcast((P, NL)))
            nc.sync.dma_start(xT[:, 1], xt[:, 1])
            nc.sync.dma_start(xT[:, 3], xt[:, 3])
            nc.scalar.dma_start(xT[:, 0, 4:8], xt[:, 0, 4:8])
            nc.scalar.dma_start(wq_sb, wq[:])
            nc.scalar.dma_start(wv_sb, wv[:])
            nc.scalar.dma_start(xT[:, 2], xt[:, 2])
            nc.scalar.dma_start(wo_sb, wo[:])

            # ---- projection / out-proj pieces (PE filler units) ----
            def proj_part(w_sb, b_sb, dest, nsub, sb, state, dcs):
                if "ps" not in state:
                    state["ps"] = psp.tile([P, 512], F32, tag="gen", bufs=2, name="ps_p")
                ps = state["ps"]
                for dc in dcs:
                    nc.tensor.matmul(
                        ps,
                        lhsT=w_sb[:, dc, nsub * P : (nsub + 1) * P],
                        rhs=xT[:, sb, dc, :],
                        start=(dc == 0),
                        stop=(dc == DC - 1),
                    )
                if dcs[-1] == DC - 1:
                    nc.vector.tensor_scalar(
                        dest[:, nsub, sb * 512 : (sb + 1) * 512],
                        ps,
                        b_sb[:, nsub : nsub + 1],
                        None,
                        ADD,
                    )

            def proj_piece(w_sb, b_sb, dest, nsub, sb):
                proj_part(w_sb, b_sb, dest, nsub, sb, {}, list(range(DC)))

            def v_piece(sc):
                ps = psp.tile([P, 512], F32, tag="gen", bufs=2, name="ps_v")
                psv = ps[:, :NL]
                for dc in range(DC):
                    nc.tensor.matmul(
                        psv,
                        lhsT=xT[:, sc // 4, dc, (sc % 4) * P : (sc % 4 + 1) * P],
                        rhs=wv_sb[:, dc, :],
                        start=(dc == 0),
                        stop=(dc == DC - 1),
                    )
                nc.vector.tensor_tensor(
                    V[:, sc],
                    psv.rearrange("p (h d) -> p h d", h=HL),
                    bv_bc.rearrange("p (h d) -> p h d", h=HL),
                    ADD,
                )

            def y_piece(qc, mb):
                ps = psp.tile([P, 512], F32, tag="gen", bufs=2, name="ps_y")
                for nch in range(2):
                    nc.tensor.matmul(
                        ps,
                        lhsT=outT[:, nch, qc * P : (qc + 1) * P],
                        rhs=wo_sb[:, nch, mb * 512 : (mb + 1) * 512],
                        start=(nch == 0),
                        stop=(nch == 1),
                    )
                yt = small.tile([P, 512], BF16, tag="yt")
                nc.vector.tensor_copy(yt, ps)
                nc.sync.dma_start(
                    y[qc * P : (qc + 1) * P, mb * 512 : (mb + 1) * 512], yt
                )

            # ---- weave schedule: step index -> filler closures ----
            fill: dict[int, list] = {}

            def put(s, fn):
                fill.setdefault(s, []).append(fn)

            def put_proj(s, w_sb, b_sb, dest, nsub, sb):
                # split into two 4-matmul halves on adjacent steps so a weave
                # never delays the next score pair beyond the ACT lookahead
                state: dict = {}
                put(s, lambda: proj_part(w_sb, b_sb, dest, nsub, sb, state, [0, 1, 2, 3]))
                put(s + 1, lambda: proj_part(w_sb, b_sb, dest, nsub, sb, state, [4, 5, 6, 7]))

            for kc in range(16):  # V just-in-time for block 0's PV (lag 3)
                put(kc + 1, lambda sc=kc: v_piece(sc))
            put_proj(2, wk_sb, bk_sb, KT, 0, 1)
            put_proj(6, wk_sb, bk_sb, KT, 0, 2)
            put_proj(10, wk_sb, bk_sb, KT, 0, 3)
            put_proj(12, wq_sb, bq_sb, QT, 0, 1)  # needed by t=1 (qb1, hp0)
            # block 1 carries the ns1 projections (needed from t=2 on)
            put_proj(17, wk_sb, bk_sb, KT, 1, 0)
            put_proj(19, wk_sb, bk_sb, KT, 1, 1)
            put_proj(22, wk_sb, bk_sb, KT, 1, 2)
            put_proj(25, wk_sb, bk_sb, KT, 1, 3)
            put_proj(28, wq_sb, bq_sb, QT, 1, 0)  # deadline t=2 (step 32)
            put_proj(33, wq_sb, bq_sb, QT, 1, 1)  # deadline t=3 (step 48)
            put_proj(49, wq_sb, bq_sb, QT, 0, 2)  # deadline t=4 (step 64)
            put_proj(65, wq_sb, bq_sb, QT, 0, 3)  # deadline t=6 (step 96)
            put_proj(70, wq_sb, bq_sb, QT, 1, 2)  # deadline t=5 (step 80)
            put_proj(97, wq_sb, bq_sb, QT, 1, 3)  # deadline t=7 (step 112)
            # y(qb) needs both (qb, hp0) and (qb, hp1) normalized:
            # q0 after t=2 (~step 53), q1 after t=3 (~step 69), q2 after t=5
            for j, base in ((0, 56), (1, 72), (2, 104)):
                for k in range(8):
                    put(base + 2 * k, lambda qc=4 * j + k // 2, mb=k % 2: y_piece(qc, mb))

            # ---- flat 128-step stream ----
            # paired-qb order: block 0 only needs the ns0 projections, and
            # each qb's two hp blocks finish within 3 blocks of each other.
            blocks = [(0, 0), (1, 0), (0, 1), (1, 1), (2, 0), (2, 1), (3, 0), (3, 1)]
            steps = [(t, kc) for t in range(len(blocks)) for kc in range(KC)]
            pv_tiles: dict = {}

            def emit_pv(ls):
                lt, lkc = steps[ls]
                lqb, lhp = blocks[lt]
                if lt not in pv_tiles:
                    pv_tiles[lt] = (
                        psp.tile([P, QBS], F32, tag="pv", bufs=1, name="pv"),
                        psp.tile([P, QBS], F32, tag="sum", bufs=1, name="sm"),
                    )
                pv, sm = pv_tiles[lt]
                st, sp = (lkc == 0), (lkc == KC - 1)
                nc.tensor.matmul(
                    pv[0:HEAD_DIM],
                    lhsT=V[:, lkc, 2 * lhp, :],
                    rhs=expT[:, lkc, 0, :],
                    start=st,
                    stop=sp,
                    skip_group_check=True,
                    tile_position=(0, 0),
                )
                nc.tensor.matmul(
                    pv[HEAD_DIM:P],
                    lhsT=V[:, lkc, 2 * lhp + 1, :],
                    rhs=expT[:, lkc, 1, :],
                    start=st,
                    stop=sp,
                    skip_group_check=True,
                    tile_position=(0, 64),
                )
                nc.tensor.matmul(
                    sm[0:HEAD_DIM],
                    lhsT=ones,
                    rhs=expT[:, lkc, 0, :],
                    start=st,
                    stop=sp,
                    skip_group_check=True,
                    tile_position=(0, 0),
                )
                nc.tensor.matmul(
                    sm[HEAD_DIM:P],
                    lhsT=ones,
                    rhs=expT[:, lkc, 1, :],
                    start=st,
                    stop=sp,
                    skip_group_check=True,
                    tile_position=(0, 64),
                )
                if lkc == KC - 1:
                    finish_block(lt)

            def finish_block(t):
                qb, hp = blocks[t]
                pv, sm = pv_tiles.pop(t)
                pvs = small.tile([P, QBS], F32, tag="pvs")
                nc.vector.tensor_copy(pvs, pv)
                smsb = small.tile([P, QBS], F32, tag="smsb")
                nc.vector.tensor_copy(smsb, sm)
                rbc = small.tile([P, QBS], F32, tag="rbc")
                if t < len(blocks) - 1:
                    nc.vector.reciprocal(rbc, smsb)
                    nc.vector.tensor_tensor(
                        outT[:, hp, qb * QBS : (qb + 1) * QBS], pvs, rbc, MULT
                    )
                else:
                    # last block: normalize in quarters, interleaving the
                    # final out-proj pieces, so the tail drain stays short
                    # and the PE never idles into a HAM re-throttle.
                    for h in range(4):
                        sl = slice(h * 128, (h + 1) * 128)
                        nc.vector.reciprocal(rbc[:, sl], smsb[:, sl])
                        nc.vector.tensor_tensor(
                            outT[:, hp, qb * QBS + h * 128 : qb * QBS + (h + 1) * 128],
                            pvs[:, sl],
                            rbc[:, sl],
                            MULT,
                        )
                        for mb in range(2):
                            y_piece(4 * qb + h, mb)

            # phase A: first KT piece + first QT piece gate the stream
            proj_piece(wk_sb, bk_sb, KT, 0, 0)
            proj_piece(wq_sb, bq_sb, QT, 0, 0)

            for s, (t, kc) in enumerate(steps):
                qb, hp = blocks[t]
                for fn in fill.get(s, ()):
                    fn()
                ps = psp.tile([P, 2, QBS], F32, tag="s", bufs=2, name="ps_s")
                nc.tensor.matmul(
                    ps[:, 0],
                    lhsT=KT[0:HEAD_DIM, hp, kc * P : (kc + 1) * P],
                    rhs=QT[0:HEAD_DIM, hp, qb * QBS : (qb + 1) * QBS],
                    start=True,
                    stop=True,
                )
                nc.tensor.matmul(
                    ps[:, 1],
                    lhsT=KT[HEAD_DIM:P, hp, kc * P : (kc + 1) * P],
                    rhs=QT[HEAD_DIM:P, hp, qb * QBS : (qb + 1) * QBS],
                    start=True,
                    stop=True,
                )
                nc.scalar.activation(expT[:, kc], ps, EXP, scale=SCALE)
                if s >= LAG:
                    emit_pv(s - LAG)
            for ls in range(len(steps) - LAG, len(steps)):
                emit_pv(ls)

    _split_excess_waits(nc)
    return nc


def _w_pmajor(W):
    """[D, NL] -> [128, DC, NL] partition-major (p = d % 128, dc = d // 128)."""
    D, n = W.shape
    return np.ascontiguousarray(
        W.reshape(D // 128, 128, n).transpose(1, 0, 2)
    ).astype(BF16_NP)


def shard_inputs(x, Wq, bq, Wk, bk, Wv, bv, Wo, bo):
    """Split full inputs into 8 per-core maps: core c -> (batch c//4, head
    group c%4).  x is transposed, bf16-converted, AND prearranged to the
    SBUF layout [p, s_block, dc, s'] on the host so device DMAs are 128
    descriptors of 8KB contiguous."""
    B, S, D = x.shape
    xts = []
    for b in range(B):
        # x[b] [S, D] -> xT [D, S] -> [dc, p, sb, s'] -> [p, sb, dc, s']
        xt = x[b].T.reshape(D // 128, 128, S // 512, 512).transpose(1, 2, 0, 3)
        xts.append(np.ascontiguousarray(xt).astype(BF16_NP))
    in_maps = []
    for c in range(8):
        b, g = c // 4, c % 4
        n0 = g * NL
        in_maps.append(
            {
                "xt": xts[b],
                "wq": _w_pmajor(Wq[:, n0 : n0 + NL]),
                "wk": _w_pmajor(Wk[:, n0 : n0 + NL]),
                "wv": _w_pmajor(Wv[:, n0 : n0 + NL]),
                "bq": np.ascontiguousarray(bq[n0 : n0 + NL]),
                "bk": np.ascontiguousarray(bk[n0 : n0 + NL]),
                "bv": np.ascontiguousarray(bv[n0 : n0 + NL]),
                "wo": _w_pmajor(Wo[n0 : n0 + NL, :]),
            }
        )
    return in_maps


_NC_CACHE = {}


def kernel(x, Wq, bq, Wk, bk, Wv, bv, Wo, bo, trace=False, tmpdir=None):
    from concourse.bass_utils import run_bass_kernel_spmd

    x = np.asarray(x, dtype=np.float32)
    args = [np.asarray(a, dtype=np.float32) for a in (Wq, bq, Wk, bk, Wv, bv, Wo, bo)]
    B, S, D = x.shape

    if S not in _NC_CACHE:
        _NC_CACHE[S] = build_bass(S)
    nc = _NC_CACHE[S]

    in_maps = shard_inputs(x, *args)
    res = run_bass_kernel_spmd(
        nc, in_maps, core_ids=list(range(8)), trace=trace, tmpdir=tmpdir
    )
    if trace:
        kernel.last_result = res
    bo_f = args[7]
    parts = [np.asarray(res.results[c]["y"]).astype(np.float32) for c in range(8)]
    out = np.empty((B, S, D), dtype=np.float32)
    for b in range(B):
        out[b] = parts[4 * b] + parts[4 * b + 1] + parts[4 * b + 2] + parts[4 * b + 3]
        out[b] += bo_f
    return out


# revision 28
# speedup vs baseline: 1.7385x; 1.0051x over previous
"""Trainium2 Bass kernel for nn_MultiHeadAttention (B=2, S=2048, D=1024, H=16).

Sharding: 8 cores = 2 (batch) x 4 (head groups of 4 heads / 256 proj dims).
Each core computes q/k/v projections for its 256-dim slice, attention for its
4 heads, and a partial out-projection y_part = attn_out @ Wo[slice].  The host
gather sums the 4 bf16 partials per batch in fp32 and adds bo.

v4 design (ACT-paced flat software pipeline):
 - x is pre-transposed, bf16-converted AND prearranged to the SBUF layout on
   the host; weights likewise.  No PE transposes, no on-chip casts, and every
   input DMA is 128 descriptors of >=2KB contiguous per partition.
 - The exp pipeline on the Activation engine is the hard floor (~16.8M exps
   per core; 128 x 1147ns ACTIVATEs = ~147us).  Everything is scheduled to
   keep ACT dense: one flat stream of 128 steps (8 blocks x 16 k-chunks);
   each step emits the 2 row-packed score matmuls for chunk kc -> one
   1024-elem ACTIVATE -> the col-packed PV + rowsum matmuls lagged LAG steps
   behind, plus at most one woven projection/out-proj piece as PE filler.
 - Rowsums via ones-matmuls (M=64 -> replicated across each head's partition
   range, aligned for the reciprocal).  (An SBUF-accumulator offload to
   DVE/GpSimd was tried and measured SLOWER: gpsimd tensor_tensor is
   ~2.1us/op and the serial chain head-of-line blocked every block.)
 - PSUM budget (8 banks): score tiles [128,2,512] x bufs=2 (4) + pv (1) +
   sm (1) + filler (2).
 - DMA: sync(SP) ring is ~180GB/s, scalar ring ~93GB/s, gpsimd ring only
   ~24GB/s (software descriptors).  Bulk inputs go on sync+scalar; ALL y
   output writes go on sync.  gpsimd does no DMA.
"""

import sys

sys.path.insert(0, "/opt/trn_rl_repo")

import ml_dtypes
import numpy as np

import concourse.bass as bass
import concourse.mybir as mybir
import concourse.tile as _tile_mod
from concourse.tile import TileContext
from concourse.vector_clock import ScopedClock


def _drain_and_barrier_split_waits(self, tick_clock, wait_clock):
    """Replacement for TileContext._drain_and_barrier.

    The walrus build in this container only accepts one sync-wait command per
    CTRL instruction; the stock tail drain carries one wait per outstanding
    proc and fails codegen with "Too many sync wait commands".  Attach the
    waits to a nop first, then redistribute the surplus onto extra nops.
    """
    carrier = self.nc.sync.nop()
    wait_clock.add_sem_waits(carrier.ins, ScopedClock({None: tick_clock.global_clock}))
    si = carrier.ins.sync_info
    if si is not None and len(si.on_wait) > 1:
        waits = list(si.on_wait)
        carrier.ins.sync_info = mybir.SyncInfo(
            on_wait=[waits[0]], on_update=list(si.on_update)
        )
        for w in waits[1:]:
            extra = self.nc.sync.nop()
            extra.ins.sync_info = mybir.SyncInfo(on_wait=[w], on_update=[])
    self.nc.sync.drain()

    self.nc.all_engine_barrier()
    assert self.sems is not None
    popped = self.nc._tile_sem_poison_stack.pop()
    assert popped is self._sem_poison
    self.nc.clear_and_free_semaphores(list(self.sems.allocated().values()))
    self.nc.all_engine_barrier()


_tile_mod.TileContext._drain_and_barrier = _drain_and_barrier_split_waits


def _split_excess_waits(nc):
    """This container's walrus accepts only ONE sync-wait command per
    instruction.  Tile emits up to 3.  Hoist all but the last wait of each
    instruction onto fresh same-engine NoOps placed directly before it --
    sound because walrus lowers DMA waits into the issuing sequencer's
    pseudo-instruction, so waits always gate the same sequencer stream."""
    ctr = 0
    for fn in nc.m.functions:
        for blk in fn.blocks:
            rewritten = []
            changed = False
            for ins in blk.instructions:
                si = ins.sync_info
                if si is not None and len(si.on_wait) > 1:
                    waits = list(si.on_wait)
                    for w in waits[:-1]:
                        nop = mybir.InstNoOp(name=f"I-wsplit-{ctr}", ins=[], outs=[])
                        ctr += 1
                        nop.engine = ins.engine
                        nop.sync_info = mybir.SyncInfo(on_wait=[w], on_update=[])
                        nc.register_instruction(nop)
                        rewritten.append(nop)
                    ins.sync_info = mybir.SyncInfo(
                        on_wait=[waits[-1]], on_update=list(si.on_update)
                    )
                    changed = True
                rewritten.append(ins)
            if changed:
                blk.instructions = rewritten
    return nc


F32 = mybir.dt.float32
BF16 = mybir.dt.bfloat16
BF16_NP = ml_dtypes.bfloat16
ADD = mybir.AluOpType.add
MULT = mybir.AluOpType.mult
EXP = mybir.ActivationFunctionType.Exp

P = 128
D_MODEL = 1024
N_HEADS = 16
HEAD_DIM = 64
SCALE = HEAD_DIM**-0.5

NL = 256  # local projection dims (4 heads x 64)
HL = 4  # local heads
QBS = 512  # q block size for attention
LAG = 3  # steps PV trails the scores/exp pipeline


def build_bass(S: int) -> bass.Bass:
    """One SPMD program; every core runs it on its own shard."""
    D = D_MODEL
    DC = D // P  # 8
    SC = S // P  # 16
    KC = S // P  # 16
    QB = S // QBS  # 4

    nc = bass.Bass()
    # all inputs host-prearranged to partition-major SBUF layouts so every
    # DMA is 128 descriptors of >=2KB contiguous per partition.
    xt = nc.declare_dram_parameter("xt", [P, QB, DC, 512], BF16, isOutput=False)
    wq = nc.declare_dram_parameter("wq", [P, DC, NL], BF16, isOutput=False)
    wk = nc.declare_dram_parameter("wk", [P, DC, NL], BF16, isOutput=False)
    wv = nc.declare_dram_parameter("wv", [P, DC, NL], BF16, isOutput=False)
    bq = nc.declare_dram_parameter("bq", [NL], F32, isOutput=False)
    bk = nc.declare_dram_parameter("bk", [NL], F32, isOutput=False)
    bv = nc.declare_dram_parameter("bv", [NL], F32, isOutput=False)
    wo = nc.declare_dram_parameter("wo", [P, 2, D], BF16, isOutput=False)
    y = nc.declare_dram_parameter("y", [S, D], BF16, isOutput=True)

    with TileContext(nc) as tc:
        with (
            tc.tile_pool(name="persist", bufs=1) as pp,
            tc.tile_pool(name="small", bufs=3) as small,
            tc.tile_pool(name="psum", bufs=1, space="PSUM") as psp,
        ):
            # ---- constants / ACT table warm-up ----
            ones = pp.tile([P, HEAD_DIM], BF16, name="ones")
            nc.vector.memset(ones, 1.0)
            warm_in = pp.tile([P, 1], F32, name="warm_in")
            nc.vector.memset(warm_in, 0.0)
            warm_out = pp.tile([P, 1], F32, name="warm_out")
            nc.scalar.activation(warm_out, warm_in, EXP)
            # HAM warm-up: dummy matmuls during the input-DMA wait so the
            # first real projection pieces run at the full 2.4GHz clock.
            junk = pp.tile([P, 512], BF16, name="junk")
            nc.vector.memset(junk, 0.0)
            ps_w = psp.tile([P, 512], F32, tag="gen", bufs=2, name="ps_warm")
            for i in range(16):
                nc.tensor.matmul(
                    ps_w[0:HEAD_DIM], lhsT=ones, rhs=junk,
                    start=(i == 0), stop=(i == 15),
                )

            # ---- persistent activations ----
            xT = pp.tile([P, QB, DC, 512], BF16, name="xT")  # s-block major
            QT = pp.tile([P, 2, S], BF16, name="QT")
            KT = pp.tile([P, 2, S], BF16, name="KT")
            V = pp.tile([P, SC, HL, HEAD_DIM], BF16, name="V")
            outT = pp.tile([P, 2, S], BF16, name="outT")
            expT = pp.tile([P, KC, 2, QBS], BF16, name="expT")

            wq_sb = pp.tile([P, DC, NL], BF16, name="wq_sb")
            wk_sb = pp.tile([P, DC, NL], BF16, name="wk_sb")
            wv_sb = pp.tile([P, DC, NL], BF16, name="wv_sb")
            wo_sb = pp.tile([P, 2, D], BF16, name="wo_sb")
            bq_sb = pp.tile([P, 2], F32, name="bq_sb")
            bk_sb = pp.tile([P, 2], F32, name="bk_sb")
            bv_bc = pp.tile([P, NL], F32, name="bv_bc")

            # x s-block 0 split across BOTH rings so the first projection
            # pieces unblock earliest; sync ring (fast) carries the rest of x.
            nc.sync.dma_start(xT[:, 0, 0:4], xt[:, 0, 0:4])
            nc.sync.dma_start(wk_sb, wk[:])
            nc.sync.dma_start(wq_sb, wq[:])
            nc.sync.dma_start(bq_sb, bq[:].rearrange("(o p) -> p o", p=P))
            nc.sync.dma_start(bk_sb, bk[:].rearrange("(o p) -> p o", p=P))
            nc.sync.dma_start(bv_bc, bv[:].unsqueeze(0).to_broadcast((P, NL)))
            nc.sync.dma_start(xT[:, 1], xt[:, 1])
            nc.sync.dma_start(xT[:, 3], xt[:, 3])
            nc.scalar.dma_start(xT[:, 0, 4:8], xt[:, 0, 4:8])
            nc.scalar.dma_start(wv_sb, wv[:])
            nc.scalar.dma_start(xT[:, 2], xt[:, 2])
            nc.scalar.dma_start(wo_sb, wo[:])

            # ---- projection / out-proj pieces (PE filler units) ----
            def proj_part(w_sb, b_sb, dest, nsub, sb, state, dcs):
                if "ps" not in state:
                    state["ps"] = psp.tile([P, 512], F32, tag="gen", bufs=2, name="ps_p")
                ps = state["ps"]
                for dc in dcs:
                    nc.tensor.matmul(
                        ps,
                        lhsT=w_sb[:, dc, nsub * P : (nsub + 1) * P],
                        rhs=xT[:, sb, dc, :],
                        start=(dc == 0),
                        stop=(dc == DC - 1),
                    )
                if dcs[-1] == DC - 1:
                    nc.vector.tensor_scalar(
                        dest[:, nsub, sb * 512 : (sb + 1) * 512],
                        ps,
                        b_sb[:, nsub : nsub + 1],
                        None,
                        ADD,
                    )

            def proj_piece(w_sb, b_sb, dest, nsub, sb):
                proj_part(w_sb, b_sb, dest, nsub, sb, {}, list(range(DC)))

            def v_piece(sc):
                ps = psp.tile([P, 512], F32, tag="gen", bufs=2, name="ps_v")
                psv = ps[:, :NL]
                for dc in range(DC):
                    nc.tensor.matmul(
                        psv,
                        lhsT=xT[:, sc // 4, dc, (sc % 4) * P : (sc % 4 + 1) * P],
                        rhs=wv_sb[:, dc, :],
                        start=(dc == 0),
                        stop=(dc == DC - 1),
                    )
                nc.vector.tensor_tensor(
                    V[:, sc],
                    psv.rearrange("p (h d) -> p h d", h=HL),
                    bv_bc.rearrange("p (h d) -> p h d", h=HL),
                    ADD,
                )

            def y_piece(qc, mb, tail=False):
                ps = psp.tile([P, 512], F32, tag="gen", bufs=2, name="ps_y")
                for nch in range(2):
                    nc.tensor.matmul(
                        ps,
                        lhsT=outT[:, nch, qc * P : (qc + 1) * P],
                        rhs=wo_sb[:, nch, mb * 512 : (mb + 1) * 512],
                        start=(nch == 0),
                        stop=(nch == 1),
                    )
                yt = small.tile([P, 512], BF16, tag="yt")
                # in the drain the ACT engine is free: splitting the casts
                # across DVE + ACT halves the serial cast chain there.
                if tail and mb == 1:
                    nc.scalar.copy(yt, ps)
                else:
                    nc.vector.tensor_copy(yt, ps)
                nc.sync.dma_start(
                    y[qc * P : (qc + 1) * P, mb * 512 : (mb + 1) * 512], yt
                )

            # ---- weave schedule: step index -> filler closures ----
            fill: dict[int, list] = {}

            def put(s, fn):
                fill.setdefault(s, []).append(fn)

            def put_proj(s, w_sb, b_sb, dest, nsub, sb):
                # split into two 4-matmul halves on adjacent steps so a weave
                # never delays the next score pair beyond the ACT lookahead
                state: dict = {}
                put(s, lambda: proj_part(w_sb, b_sb, dest, nsub, sb, state, [0, 1, 2, 3]))
                put(s + 1, lambda: proj_part(w_sb, b_sb, dest, nsub, sb, state, [4, 5, 6, 7]))

            for kc in range(16):  # V just-in-time for block 0's PV (lag 3)
                put(kc + 1, lambda sc=kc: v_piece(sc))
            put_proj(2, wk_sb, bk_sb, KT, 0, 1)
            put_proj(6, wk_sb, bk_sb, KT, 0, 2)
            put_proj(10, wk_sb, bk_sb, KT, 0, 3)
            put_proj(12, wq_sb, bq_sb, QT, 0, 1)  # needed by t=1 (qb1, hp0)
            # block 1 carries the ns1 projections (needed from t=2 on)
            put_proj(17, wk_sb, bk_sb, KT, 1, 0)
            put_proj(19, wk_sb, bk_sb, KT, 1, 1)
            put_proj(22, wk_sb, bk_sb, KT, 1, 2)
            put_proj(25, wk_sb, bk_sb, KT, 1, 3)
            put_proj(28, wq_sb, bq_sb, QT, 1, 0)  # deadline t=2 (step 32)
            put_proj(33, wq_sb, bq_sb, QT, 1, 1)  # deadline t=3 (step 48)
            put_proj(49, wq_sb, bq_sb, QT, 0, 2)  # deadline t=4 (step 64)
            put_proj(76, wq_sb, bq_sb, QT, 1, 2)  # deadline t=5 (step 80)
            put_proj(90, wq_sb, bq_sb, QT, 0, 3)  # deadline t=6 (step 96)
            put_proj(97, wq_sb, bq_sb, QT, 1, 3)  # deadline t=7 (step 112)
            # y(qb) needs both (qb, hp0) and (qb, hp1) normalized:
            # q0 after t=2 (~step 53), q1 after t=3 (~step 69), q2 after t=5
            for j, base in ((0, 56), (1, 72), (2, 104)):
                for k in range(8):
                    put(base + 2 * k, lambda qc=4 * j + k // 2, mb=k % 2: y_piece(qc, mb))

            # ---- flat 128-step stream ----
            # paired-qb order: block 0 only needs the ns0 projections, and
            # each qb's two hp blocks finish within 3 blocks of each other.
            blocks = [(0, 0), (1, 0), (0, 1), (1, 1), (2, 0), (2, 1), (3, 0), (3, 1)]
            steps = [(t, kc) for t in range(len(blocks)) for kc in range(KC)]
            pv_tiles: dict = {}

            def emit_pv(ls):
                lt, lkc = steps[ls]
                lqb, lhp = blocks[lt]
                if lt not in pv_tiles:
                    pv_tiles[lt] = (
                        psp.tile([P, QBS], F32, tag="pv", bufs=1, name="pv"),
                        psp.tile([P, QBS], F32, tag="sum", bufs=1, name="sm"),
                    )
                pv, sm = pv_tiles[lt]
                st, sp = (lkc == 0), (lkc == KC - 1)
                nc.tensor.matmul(
                    pv[0:HEAD_DIM],
                    lhsT=V[:, lkc, 2 * lhp, :],
                    rhs=expT[:, lkc, 0, :],
                    start=st,
                    stop=sp,
                    skip_group_check=True,
                    tile_position=(0, 0),
                )
                nc.tensor.matmul(
                    pv[HEAD_DIM:P],
                    lhsT=V[:, lkc, 2 * lhp + 1, :],
                    rhs=expT[:, lkc, 1, :],
                    start=st,
                    stop=sp,
                    skip_group_check=True,
                    tile_position=(0, 64),
                )
                nc.tensor.matmul(
                    sm[0:HEAD_DIM],
                    lhsT=ones,
                    rhs=expT[:, lkc, 0, :],
                    start=st,
                    stop=sp,
                    skip_group_check=True,
                    tile_position=(0, 0),
                )
                nc.tensor.matmul(
                    sm[HEAD_DIM:P],
                    lhsT=ones,
                    rhs=expT[:, lkc, 1, :],
                    start=st,
                    stop=sp,
                    skip_group_check=True,
                    tile_position=(0, 64),
                )
                if lkc == KC - 1:
                    finish_block(lt)

            def finish_block(t):
                qb, hp = blocks[t]
                pv, sm = pv_tiles.pop(t)
                pvs = small.tile([P, QBS], F32, tag="pvs")
                nc.vector.tensor_copy(pvs, pv)
                smsb = small.tile([P, QBS], F32, tag="smsb")
                nc.vector.tensor_copy(smsb, sm)
                rbc = small.tile([P, QBS], F32, tag="rbc")
                if t < len(blocks) - 1:
                    nc.vector.reciprocal(rbc, smsb)
                    nc.vector.tensor_tensor(
                        outT[:, hp, qb * QBS : (qb + 1) * QBS], pvs, rbc, MULT
                    )
                else:
                    # last block: normalize in quarters, interleaving the
                    # final out-proj pieces, so the tail drain stays short
                    # and the PE never idles into a HAM re-throttle.
                    for h in range(4):
                        sl = slice(h * 128, (h + 1) * 128)
                        nc.vector.reciprocal(rbc[:, sl], smsb[:, sl])
                        nc.vector.tensor_tensor(
                            outT[:, hp, qb * QBS + h * 128 : qb * QBS + (h + 1) * 128],
                            pvs[:, sl],
                            rbc[:, sl],
                            MULT,
                        )
                        for mb in range(2):
                            y_piece(4 * qb + h, mb, tail=True)

            # phase A: first KT piece + first QT piece gate the stream
            proj_piece(wk_sb, bk_sb, KT, 0, 0)
            proj_piece(wq_sb, bq_sb, QT, 0, 0)

            for s, (t, kc) in enumerate(steps):
                qb, hp = blocks[t]
                for fn in fill.get(s, ()):
                    fn()
                ps = psp.tile([P, 2, QBS], F32, tag="s", bufs=2, name="ps_s")
                nc.tensor.matmul(
                    ps[:, 0],
                    lhsT=KT[0:HEAD_DIM, hp, kc * P : (kc + 1) * P],
                    rhs=QT[0:HEAD_DIM, hp, qb * QBS : (qb + 1) * QBS],
                    start=True,
                    stop=True,
                )
                nc.tensor.matmul(
                    ps[:, 1],
                    lhsT=KT[HEAD_DIM:P, hp, kc * P : (kc + 1) * P],
                    rhs=QT[HEAD_DIM:P, hp, qb * QBS : (qb + 1) * QBS],
                    start=True,
                    stop=True,
                )
                nc.scalar.activation(expT[:, kc], ps, EXP, scale=SCALE)
                if s >= LAG:
                    emit_pv(s - LAG)
            for ls in range(len(steps) - LAG, len(steps)):
                emit_pv(ls)

    _split_excess_waits(nc)
    return nc


def _w_pmajor(W):
    """[D, NL] -> [128, DC, NL] partition-major (p = d % 128, dc = d // 128)."""
    D, n = W.shape
    return np.ascontiguousarray(
        W.reshape(D // 128, 128, n).transpose(1, 0, 2)
    ).astype(BF16_NP)


def shard_inputs(x, Wq, bq, Wk, bk, Wv, bv, Wo, bo):
    """Split full inputs into 8 per-core maps: core c -> (batch c//4, head
    group c%4).  x is transposed, bf16-converted, AND prearranged to the
    SBUF layout [p, s_block, dc, s'] on the host so device DMAs are 128
    descriptors of 8KB contiguous."""
    B, S, D = x.shape
    xts = []
    for b in range(B):
        # x[b] [S, D] -> xT [D, S] -> [dc, p, sb, s'] -> [p, sb, dc, s']
        xt = x[b].T.reshape(D // 128, 128, S // 512, 512).transpose(1, 2, 0, 3)
        xts.append(np.ascontiguousarray(xt).astype(BF16_NP))
    in_maps = []
    for c in range(8):
        b, g = c // 4, c % 4
        n0 = g * NL
        in_maps.append(
            {
                "xt": xts[b],
                "wq": _w_pmajor(Wq[:, n0 : n0 + NL]),
                "wk": _w_pmajor(Wk[:, n0 : n0 + NL]),
                "wv": _w_pmajor(Wv[:, n0 : n0 + NL]),
                "bq": np.ascontiguousarray(bq[n0 : n0 + NL]),
                "bk": np.ascontiguousarray(bk[n0 : n0 + NL]),
                "bv": np.ascontiguousarray(bv[n0 : n0 + NL]),
                "wo": _w_pmajor(Wo[n0 : n0 + NL, :]),
            }
        )
    return in_maps


_NC_CACHE = {}


def kernel(x, Wq, bq, Wk, bk, Wv, bv, Wo, bo, trace=False, tmpdir=None):
    from concourse.bass_utils import run_bass_kernel_spmd

    x = np.asarray(x, dtype=np.float32)
    args = [np.asarray(a, dtype=np.float32) for a in (Wq, bq, Wk, bk, Wv, bv, Wo, bo)]
    B, S, D = x.shape

    if S not in _NC_CACHE:
        _NC_CACHE[S] = build_bass(S)
    nc = _NC_CACHE[S]

    in_maps = shard_inputs(x, *args)
    res = run_bass_kernel_spmd(
        nc, in_maps, core_ids=list(range(8)), trace=trace, tmpdir=tmpdir
    )
    if trace:
        kernel.last_result = res
    bo_f = args[7]
    parts = [np.asarray(res.results[c]["y"]).astype(np.float32) for c in range(8)]
    out = np.empty((B, S, D), dtype=np.float32)
    for b in range(B):
        out[b] = parts[4 * b] + parts[4 * b + 1] + parts[4 * b + 2] + parts[4 * b + 3]
        out[b] += bo_f
    return out


# revision 30
# speedup vs baseline: 1.7403x; 1.0011x over previous
"""Trainium2 Bass kernel for nn_MultiHeadAttention (B=2, S=2048, D=1024, H=16).

Sharding: 8 cores = 2 (batch) x 4 (head groups of 4 heads / 256 proj dims).
Each core computes q/k/v projections for its 256-dim slice, attention for its
4 heads, and a partial out-projection y_part = attn_out @ Wo[slice].  The host
gather sums the 4 bf16 partials per batch in fp32 and adds bo.

v4 design (ACT-paced flat software pipeline):
 - x is pre-transposed, bf16-converted AND prearranged to the SBUF layout on
   the host; weights likewise.  No PE transposes, no on-chip casts, and every
   input DMA is 128 descriptors of >=2KB contiguous per partition.
 - The exp pipeline on the Activation engine is the hard floor (~16.8M exps
   per core; 128 x 1147ns ACTIVATEs = ~147us).  Everything is scheduled to
   keep ACT dense: one flat stream of 128 steps (8 blocks x 16 k-chunks);
   each step emits the 2 row-packed score matmuls for chunk kc -> one
   1024-elem ACTIVATE -> the col-packed PV + rowsum matmuls lagged LAG steps
   behind, plus at most one woven projection/out-proj piece as PE filler.
 - Rowsums via ones-matmuls (M=64 -> replicated across each head's partition
   range, aligned for the reciprocal).  (An SBUF-accumulator offload to
   DVE/GpSimd was tried and measured SLOWER: gpsimd tensor_tensor is
   ~2.1us/op and the serial chain head-of-line blocked every block.)
 - PSUM budget (8 banks): score tiles [128,2,512] x bufs=2 (4) + pv (1) +
   sm (1) + filler (2).
 - DMA: sync(SP) ring is ~180GB/s, scalar ring ~93GB/s, gpsimd ring only
   ~24GB/s (software descriptors).  Bulk inputs go on sync+scalar; ALL y
   output writes go on sync.  gpsimd does no DMA.
"""

import sys

sys.path.insert(0, "/opt/trn_rl_repo")

import ml_dtypes
import numpy as np

import concourse.bass as bass
import concourse.mybir as mybir
import concourse.tile as _tile_mod
from concourse.tile import TileContext
from concourse.vector_clock import ScopedClock


def _drain_and_barrier_split_waits(self, tick_clock, wait_clock):
    """Replacement for TileContext._drain_and_barrier.

    The walrus build in this container only accepts one sync-wait command per
    CTRL instruction; the stock tail drain carries one wait per outstanding
    proc and fails codegen with "Too many sync wait commands".  Attach the
    waits to a nop first, then redistribute the surplus onto extra nops.
    """
    carrier = self.nc.sync.nop()
    wait_clock.add_sem_waits(carrier.ins, ScopedClock({None: tick_clock.global_clock}))
    si = carrier.ins.sync_info
    if si is not None and len(si.on_wait) > 1:
        waits = list(si.on_wait)
        carrier.ins.sync_info = mybir.SyncInfo(
            on_wait=[waits[0]], on_update=list(si.on_update)
        )
        for w in waits[1:]:
            extra = self.nc.sync.nop()
            extra.ins.sync_info = mybir.SyncInfo(on_wait=[w], on_update=[])
    self.nc.sync.drain()

    self.nc.all_engine_barrier()
    assert self.sems is not None
    popped = self.nc._tile_sem_poison_stack.pop()
    assert popped is self._sem_poison
    self.nc.clear_and_free_semaphores(list(self.sems.allocated().values()))
    self.nc.all_engine_barrier()


_tile_mod.TileContext._drain_and_barrier = _drain_and_barrier_split_waits


def _split_excess_waits(nc):
    """This container's walrus accepts only ONE sync-wait command per
    instruction.  Tile emits up to 3.  Hoist all but the last wait of each
    instruction onto fresh same-engine NoOps placed directly before it --
    sound because walrus lowers DMA waits into the issuing sequencer's
    pseudo-instruction, so waits always gate the same sequencer stream."""
    ctr = 0
    for fn in nc.m.functions:
        for blk in fn.blocks:
            rewritten = []
            changed = False
            for ins in blk.instructions:
                si = ins.sync_info
                if si is not None and len(si.on_wait) > 1:
                    waits = list(si.on_wait)
                    for w in waits[:-1]:
                        nop = mybir.InstNoOp(name=f"I-wsplit-{ctr}", ins=[], outs=[])
                        ctr += 1
                        nop.engine = ins.engine
                        nop.sync_info = mybir.SyncInfo(on_wait=[w], on_update=[])
                        nc.register_instruction(nop)
                        rewritten.append(nop)
                    ins.sync_info = mybir.SyncInfo(
                        on_wait=[waits[-1]], on_update=list(si.on_update)
                    )
                    changed = True
                rewritten.append(ins)
            if changed:
                blk.instructions = rewritten
    return nc


F32 = mybir.dt.float32
BF16 = mybir.dt.bfloat16
BF16_NP = ml_dtypes.bfloat16
ADD = mybir.AluOpType.add
MULT = mybir.AluOpType.mult
EXP = mybir.ActivationFunctionType.Exp

P = 128
D_MODEL = 1024
N_HEADS = 16
HEAD_DIM = 64
SCALE = HEAD_DIM**-0.5

NL = 256  # local projection dims (4 heads x 64)
HL = 4  # local heads
QBS = 512  # q block size for attention
LAG = 3  # steps PV trails the scores/exp pipeline


def build_bass(S: int) -> bass.Bass:
    """One SPMD program; every core runs it on its own shard."""
    D = D_MODEL
    DC = D // P  # 8
    SC = S // P  # 16
    KC = S // P  # 16
    QB = S // QBS  # 4

    nc = bass.Bass()
    # all inputs host-prearranged to partition-major SBUF layouts so every
    # DMA is 128 descriptors of >=2KB contiguous per partition.
    xt = nc.declare_dram_parameter("xt", [P, QB, DC, 512], BF16, isOutput=False)
    wq = nc.declare_dram_parameter("wq", [P, DC, NL], BF16, isOutput=False)
    wk = nc.declare_dram_parameter("wk", [P, DC, NL], BF16, isOutput=False)
    wv = nc.declare_dram_parameter("wv", [P, DC, NL], BF16, isOutput=False)
    bq = nc.declare_dram_parameter("bq", [NL], F32, isOutput=False)
    bk = nc.declare_dram_parameter("bk", [NL], F32, isOutput=False)
    bv = nc.declare_dram_parameter("bv", [NL], F32, isOutput=False)
    wo = nc.declare_dram_parameter("wo", [P, 2, D], BF16, isOutput=False)
    y = nc.declare_dram_parameter("y", [S, D], BF16, isOutput=True)

    with TileContext(nc) as tc:
        with (
            tc.tile_pool(name="persist", bufs=1) as pp,
            tc.tile_pool(name="small", bufs=3) as small,
            tc.tile_pool(name="psum", bufs=1, space="PSUM") as psp,
        ):
            # ---- constants / ACT table warm-up ----
            ones = pp.tile([P, HEAD_DIM], BF16, name="ones")
            nc.vector.memset(ones, 1.0)
            warm_in = pp.tile([P, 1], F32, name="warm_in")
            nc.vector.memset(warm_in, 0.0)
            warm_out = pp.tile([P, 1], F32, name="warm_out")
            nc.scalar.activation(warm_out, warm_in, EXP)
            # HAM warm-up: dummy matmuls during the input-DMA wait so the
            # first real projection pieces run at the full 2.4GHz clock.
            junk = pp.tile([P, 512], BF16, name="junk")
            nc.vector.memset(junk, 0.0)
            ps_w = psp.tile([P, 512], F32, tag="gen", bufs=2, name="ps_warm")
            for i in range(18):
                nc.tensor.matmul(
                    ps_w[0:HEAD_DIM], lhsT=ones, rhs=junk,
                    start=(i == 0), stop=(i == 17),
                )

            # ---- persistent activations ----
            xT = pp.tile([P, QB, DC, 512], BF16, name="xT")  # s-block major
            QT = pp.tile([P, 2, S], BF16, name="QT")
            KT = pp.tile([P, 2, S], BF16, name="KT")
            V = pp.tile([P, SC, HL, HEAD_DIM], BF16, name="V")
            outT = pp.tile([P, 2, S], BF16, name="outT")
            expT = pp.tile([P, KC, 2, QBS], BF16, name="expT")

            wq_sb = pp.tile([P, DC, NL], BF16, name="wq_sb")
            wk_sb = pp.tile([P, DC, NL], BF16, name="wk_sb")
            wv_sb = pp.tile([P, DC, NL], BF16, name="wv_sb")
            wo_sb = pp.tile([P, 2, D], BF16, name="wo_sb")
            bq_sb = pp.tile([P, 2], F32, name="bq_sb")
            bk_sb = pp.tile([P, 2], F32, name="bk_sb")
            bv_bc = pp.tile([P, NL], F32, name="bv_bc")

            # x s-block 0 split across BOTH rings so the first projection
            # pieces unblock earliest; sync ring (fast) carries the rest of x.
            nc.sync.dma_start(xT[:, 0, 0:2], xt[:, 0, 0:2])
            nc.sync.dma_start(wk_sb, wk[:])
            nc.sync.dma_start(wq_sb, wq[:])
            nc.sync.dma_start(bq_sb, bq[:].rearrange("(o p) -> p o", p=P))
            nc.sync.dma_start(bk_sb, bk[:].rearrange("(o p) -> p o", p=P))
            nc.sync.dma_start(bv_bc, bv[:].unsqueeze(0).to_broadcast((P, NL)))
            nc.sync.dma_start(xT[:, 1], xt[:, 1])
            nc.sync.dma_start(xT[:, 3], xt[:, 3])
            nc.scalar.dma_start(xT[:, 0, 2:4], xt[:, 0, 2:4])
            nc.scalar.dma_start(xT[:, 0, 4:8], xt[:, 0, 4:8])
            nc.scalar.dma_start(wv_sb, wv[:])
            nc.scalar.dma_start(xT[:, 2], xt[:, 2])
            nc.scalar.dma_start(wo_sb, wo[:])

            # ---- projection / out-proj pieces (PE filler units) ----
            def proj_part(w_sb, b_sb, dest, nsub, sb, state, dcs):
                if "ps" not in state:
                    state["ps"] = psp.tile([P, 512], F32, tag="gen", bufs=2, name="ps_p")
                ps = state["ps"]
                for dc in dcs:
                    nc.tensor.matmul(
                        ps,
                        lhsT=w_sb[:, dc, nsub * P : (nsub + 1) * P],
                        rhs=xT[:, sb, dc, :],
                        start=(dc == 0),
                        stop=(dc == DC - 1),
                    )
                if dcs[-1] == DC - 1:
                    nc.vector.tensor_scalar(
                        dest[:, nsub, sb * 512 : (sb + 1) * 512],
                        ps,
                        b_sb[:, nsub : nsub + 1],
                        None,
                        ADD,
                    )

            def proj_piece(w_sb, b_sb, dest, nsub, sb):
                proj_part(w_sb, b_sb, dest, nsub, sb, {}, list(range(DC)))

            def v_piece(sc):
                ps = psp.tile([P, 512], F32, tag="gen", bufs=2, name="ps_v")
                psv = ps[:, :NL]
                for dc in range(DC):
                    nc.tensor.matmul(
                        psv,
                        lhsT=xT[:, sc // 4, dc, (sc % 4) * P : (sc % 4 + 1) * P],
                        rhs=wv_sb[:, dc, :],
                        start=(dc == 0),
                        stop=(dc == DC - 1),
                    )
                nc.vector.tensor_tensor(
                    V[:, sc],
                    psv.rearrange("p (h d) -> p h d", h=HL),
                    bv_bc.rearrange("p (h d) -> p h d", h=HL),
                    ADD,
                )

            def y_piece(qc, mb, tail=False):
                ps = psp.tile([P, 512], F32, tag="gen", bufs=2, name="ps_y")
                for nch in range(2):
                    nc.tensor.matmul(
                        ps,
                        lhsT=outT[:, nch, qc * P : (qc + 1) * P],
                        rhs=wo_sb[:, nch, mb * 512 : (mb + 1) * 512],
                        start=(nch == 0),
                        stop=(nch == 1),
                    )
                yt = small.tile([P, 512], BF16, tag="yt")
                # in the drain the ACT engine is free: splitting the casts
                # across DVE + ACT halves the serial cast chain there.
                if tail and mb == 1:
                    nc.scalar.copy(yt, ps)
                else:
                    nc.vector.tensor_copy(yt, ps)
                nc.sync.dma_start(
                    y[qc * P : (qc + 1) * P, mb * 512 : (mb + 1) * 512], yt
                )

            # ---- weave schedule: step index -> filler closures ----
            fill: dict[int, list] = {}

            def put(s, fn):
                fill.setdefault(s, []).append(fn)

            def put_proj(s, w_sb, b_sb, dest, nsub, sb):
                # split into two 4-matmul halves on adjacent steps so a weave
                # never delays the next score pair beyond the ACT lookahead
                state: dict = {}
                put(s, lambda: proj_part(w_sb, b_sb, dest, nsub, sb, state, [0, 1, 2, 3]))
                put(s + 1, lambda: proj_part(w_sb, b_sb, dest, nsub, sb, state, [4, 5, 6, 7]))

            for kc in range(16):  # V just-in-time for block 0's PV (lag 3)
                put(kc + 1, lambda sc=kc: v_piece(sc))
            put_proj(2, wk_sb, bk_sb, KT, 0, 1)
            put_proj(6, wk_sb, bk_sb, KT, 0, 2)
            put_proj(10, wk_sb, bk_sb, KT, 0, 3)
            put_proj(12, wq_sb, bq_sb, QT, 0, 1)  # needed by t=1 (qb1, hp0)
            # block 1 carries the ns1 projections (needed from t=2 on)
            put_proj(17, wk_sb, bk_sb, KT, 1, 0)
            put_proj(19, wk_sb, bk_sb, KT, 1, 1)
            put_proj(22, wk_sb, bk_sb, KT, 1, 2)
            put_proj(25, wk_sb, bk_sb, KT, 1, 3)
            put_proj(28, wq_sb, bq_sb, QT, 1, 0)  # deadline t=2 (step 32)
            put_proj(33, wq_sb, bq_sb, QT, 1, 1)  # deadline t=3 (step 48)
            put_proj(49, wq_sb, bq_sb, QT, 0, 2)  # deadline t=4 (step 64)
            put_proj(76, wq_sb, bq_sb, QT, 1, 2)  # deadline t=5 (step 80)
            put_proj(90, wq_sb, bq_sb, QT, 0, 3)  # deadline t=6 (step 96)
            put_proj(97, wq_sb, bq_sb, QT, 1, 3)  # deadline t=7 (step 112)
            # y(qb) needs both (qb, hp0) and (qb, hp1) normalized:
            # q0 after t=2 (~step 53), q1 after t=3 (~step 69), q2 after t=5
            for j, base in ((0, 56), (1, 72), (2, 104)):
                for k in range(8):
                    put(base + 2 * k, lambda qc=4 * j + k // 2, mb=k % 2: y_piece(qc, mb))

            # ---- flat 128-step stream ----
            # paired-qb order: block 0 only needs the ns0 projections, and
            # each qb's two hp blocks finish within 3 blocks of each other.
            blocks = [(0, 0), (1, 0), (0, 1), (1, 1), (2, 0), (2, 1), (3, 0), (3, 1)]
            steps = [(t, kc) for t in range(len(blocks)) for kc in range(KC)]
            pv_tiles: dict = {}

            def emit_pv(ls):
                lt, lkc = steps[ls]
                lqb, lhp = blocks[lt]
                if lt not in pv_tiles:
                    pv_tiles[lt] = (
                        psp.tile([P, QBS], F32, tag="pv", bufs=1, name="pv"),
                        psp.tile([P, QBS], F32, tag="sum", bufs=1, name="sm"),
                    )
                pv, sm = pv_tiles[lt]
                st, sp = (lkc == 0), (lkc == KC - 1)
                nc.tensor.matmul(
                    pv[0:HEAD_DIM],
                    lhsT=V[:, lkc, 2 * lhp, :],
                    rhs=expT[:, lkc, 0, :],
                    start=st,
                    stop=sp,
                    skip_group_check=True,
                    tile_position=(0, 0),
                )
                nc.tensor.matmul(
                    pv[HEAD_DIM:P],
                    lhsT=V[:, lkc, 2 * lhp + 1, :],
                    rhs=expT[:, lkc, 1, :],
                    start=st,
                    stop=sp,
                    skip_group_check=True,
                    tile_position=(0, 64),
                )
                nc.tensor.matmul(
                    sm[0:HEAD_DIM],
                    lhsT=ones,
                    rhs=expT[:, lkc, 0, :],
                    start=st,
                    stop=sp,
                    skip_group_check=True,
                    tile_position=(0, 0),
                )
                nc.tensor.matmul(
                    sm[HEAD_DIM:P],
                    lhsT=ones,
                    rhs=expT[:, lkc, 1, :],
                    start=st,
                    stop=sp,
                    skip_group_check=True,
                    tile_position=(0, 64),
                )
                if lkc == KC - 1:
                    finish_block(lt)

            def finish_block(t):
                qb, hp = blocks[t]
                pv, sm = pv_tiles.pop(t)
                pvs = small.tile([P, QBS], F32, tag="pvs")
                nc.vector.tensor_copy(pvs, pv)
                smsb = small.tile([P, QBS], F32, tag="smsb")
                nc.vector.tensor_copy(smsb, sm)
                rbc = small.tile([P, QBS], F32, tag="rbc")
                if t < len(blocks) - 1:
                    nc.vector.reciprocal(rbc, smsb)
                    nc.vector.tensor_tensor(
                        outT[:, hp, qb * QBS : (qb + 1) * QBS], pvs, rbc, MULT
                    )
                else:
                    # last block: normalize in quarters, interleaving the
                    # final out-proj pieces, so the tail drain stays short
                    # and the PE never idles into a HAM re-throttle.
                    for h in range(4):
                        sl = slice(h * 128, (h + 1) * 128)
                        nc.vector.reciprocal(rbc[:, sl], smsb[:, sl])
                        nc.vector.tensor_tensor(
                            outT[:, hp, qb * QBS + h * 128 : qb * QBS + (h + 1) * 128],
                            pvs[:, sl],
                            rbc[:, sl],
                            MULT,
                        )
                        for mb in range(2):
                            y_piece(4 * qb + h, mb, tail=True)

            # phase A: first KT piece + first QT piece gate the stream
            proj_piece(wk_sb, bk_sb, KT, 0, 0)
            proj_piece(wq_sb, bq_sb, QT, 0, 0)

            for s, (t, kc) in enumerate(steps):
                qb, hp = blocks[t]
                for fn in fill.get(s, ()):
                    fn()
                ps = psp.tile([P, 2, QBS], F32, tag="s", bufs=2, name="ps_s")
                nc.tensor.matmul(
                    ps[:, 0],
                    lhsT=KT[0:HEAD_DIM, hp, kc * P : (kc + 1) * P],
                    rhs=QT[0:HEAD_DIM, hp, qb * QBS : (qb + 1) * QBS],
                    start=True,
                    stop=True,
                )
                nc.tensor.matmul(
                    ps[:, 1],
                    lhsT=KT[HEAD_DIM:P, hp, kc * P : (kc + 1) * P],
                    rhs=QT[HEAD_DIM:P, hp, qb * QBS : (qb + 1) * QBS],
                    start=True,
                    stop=True,
                )
                nc.scalar.activation(expT[:, kc], ps, EXP, scale=SCALE)
                if s >= LAG:
                    emit_pv(s - LAG)
            for ls in range(len(steps) - LAG, len(steps)):
                emit_pv(ls)

    _split_excess_waits(nc)
    return nc


def _w_pmajor(W):
    """[D, NL] -> [128, DC, NL] partition-major (p = d % 128, dc = d // 128)."""
    D, n = W.shape
    return np.ascontiguousarray(
        W.reshape(D // 128, 128, n).transpose(1, 0, 2)
    ).astype(BF16_NP)


def shard_inputs(x, Wq, bq, Wk, bk, Wv, bv, Wo, bo):
    """Split full inputs into 8 per-core maps: core c -> (batch c//4, head
    group c%4).  x is transposed, bf16-converted, AND prearranged to the
    SBUF layout [p, s_block, dc, s'] on the host so device DMAs are 128
    descriptors of 8KB contiguous."""
    B, S, D = x.shape
    xts = []
    for b in range(B):
        # x[b] [S, D] -> xT [D, S] -> [dc, p, sb, s'] -> [p, sb, dc, s']
        xt = x[b].T.reshape(D // 128, 128, S // 512, 512).transpose(1, 2, 0, 3)
        xts.append(np.ascontiguousarray(xt).astype(BF16_NP))
    in_maps = []
    for c in range(8):
        b, g = c // 4, c % 4
        n0 = g * NL
        in_maps.append(
            {
                "xt": xts[b],
                "wq": _w_pmajor(Wq[:, n0 : n0 + NL]),
                "wk": _w_pmajor(Wk[:, n0 : n0 + NL]),
                "wv": _w_pmajor(Wv[:, n0 : n0 + NL]),
                "bq": np.ascontiguousarray(bq[n0 : n0 + NL]),
                "bk": np.ascontiguousarray(bk[n0 : n0 + NL]),
                "bv": np.ascontiguousarray(bv[n0 : n0 + NL]),
                "wo": _w_pmajor(Wo[n0 : n0 + NL, :]),
            }
        )
    return in_maps


_NC_CACHE = {}


def kernel(x, Wq, bq, Wk, bk, Wv, bv, Wo, bo, trace=False, tmpdir=None):
    from concourse.bass_utils import run_bass_kernel_spmd

    x = np.asarray(x, dtype=np.float32)
    args = [np.asarray(a, dtype=np.float32) for a in (Wq, bq, Wk, bk, Wv, bv, Wo, bo)]
    B, S, D = x.shape

    if S not in _NC_CACHE:
        _NC_CACHE[S] = build_bass(S)
    nc = _NC_CACHE[S]

    in_maps = shard_inputs(x, *args)
    res = run_bass_kernel_spmd(
        nc, in_maps, core_ids=list(range(8)), trace=trace, tmpdir=tmpdir
    )
    if trace:
        kernel.last_result = res
    bo_f = args[7]
    parts = [np.asarray(res.results[c]["y"]).astype(np.float32) for c in range(8)]
    out = np.empty((B, S, D), dtype=np.float32)
    for b in range(B):
        out[b] = parts[4 * b] + parts[4 * b + 1] + parts[4 * b + 2] + parts[4 * b + 3]
        out[b] += bo_f
    return out


# revision 32
# speedup vs baseline: 1.7476x; 1.0042x over previous
"""Trainium2 Bass kernel for nn_MultiHeadAttention (B=2, S=2048, D=1024, H=16).

Sharding: 8 cores = 2 (batch) x 4 (head groups of 4 heads / 256 proj dims).
Each core computes q/k/v projections for its 256-dim slice, attention for its
4 heads, and a partial out-projection y_part = attn_out @ Wo[slice].  The host
gather sums the 4 bf16 partials per batch in fp32 and adds bo.

v4 design (ACT-paced flat software pipeline):
 - x is pre-transposed, bf16-converted AND prearranged to the SBUF layout on
   the host; weights likewise.  No PE transposes, no on-chip casts, and every
   input DMA is 128 descriptors of >=2KB contiguous per partition.
 - The exp pipeline on the Activation engine is the hard floor (~16.8M exps
   per core; 128 x 1147ns ACTIVATEs = ~147us).  Everything is scheduled to
   keep ACT dense: one flat stream of 128 steps (8 blocks x 16 k-chunks);
   each step emits the 2 row-packed score matmuls for chunk kc -> one
   1024-elem ACTIVATE -> the col-packed PV + rowsum matmuls lagged LAG steps
   behind, plus at most one woven projection/out-proj piece as PE filler.
 - Rowsums via ones-matmuls (M=64 -> replicated across each head's partition
   range, aligned for the reciprocal).  (An SBUF-accumulator offload to
   DVE/GpSimd was tried and measured SLOWER: gpsimd tensor_tensor is
   ~2.1us/op and the serial chain head-of-line blocked every block.)
 - PSUM budget (8 banks): score tiles [128,2,512] x bufs=2 (4) + pv (1) +
   sm (1) + filler (2).
 - DMA: sync(SP) ring is ~180GB/s, scalar ring ~93GB/s, gpsimd ring only
   ~24GB/s (software descriptors).  Bulk inputs go on sync+scalar; ALL y
   output writes go on sync.  gpsimd does no DMA.
"""

import sys

sys.path.insert(0, "/opt/trn_rl_repo")

import ml_dtypes
import numpy as np

import concourse.bass as bass
import concourse.mybir as mybir
import concourse.tile as _tile_mod
from concourse.tile import TileContext
from concourse.vector_clock import ScopedClock


def _drain_and_barrier_split_waits(self, tick_clock, wait_clock):
    """Replacement for TileContext._drain_and_barrier.

    The walrus build in this container only accepts one sync-wait command per
    CTRL instruction; the stock tail drain carries one wait per outstanding
    proc and fails codegen with "Too many sync wait commands".  Attach the
    waits to a nop first, then redistribute the surplus onto extra nops.
    """
    carrier = self.nc.sync.nop()
    wait_clock.add_sem_waits(carrier.ins, ScopedClock({None: tick_clock.global_clock}))
    si = carrier.ins.sync_info
    if si is not None and len(si.on_wait) > 1:
        waits = list(si.on_wait)
        carrier.ins.sync_info = mybir.SyncInfo(
            on_wait=[waits[0]], on_update=list(si.on_update)
        )
        for w in waits[1:]:
            extra = self.nc.sync.nop()
            extra.ins.sync_info = mybir.SyncInfo(on_wait=[w], on_update=[])
    self.nc.sync.drain()

    self.nc.all_engine_barrier()
    assert self.sems is not None
    popped = self.nc._tile_sem_poison_stack.pop()
    assert popped is self._sem_poison
    self.nc.clear_and_free_semaphores(list(self.sems.allocated().values()))
    self.nc.all_engine_barrier()


_tile_mod.TileContext._drain_and_barrier = _drain_and_barrier_split_waits


def _split_excess_waits(nc):
    """This container's walrus accepts only ONE sync-wait command per
    instruction.  Tile emits up to 3.  Hoist all but the last wait of each
    instruction onto fresh same-engine NoOps placed directly before it --
    sound because walrus lowers DMA waits into the issuing sequencer's
    pseudo-instruction, so waits always gate the same sequencer stream."""
    ctr = 0
    for fn in nc.m.functions:
        for blk in fn.blocks:
            rewritten = []
            changed = False
            for ins in blk.instructions:
                si = ins.sync_info
                if si is not None and len(si.on_wait) > 1:
                    waits = list(si.on_wait)
                    for w in waits[:-1]:
                        nop = mybir.InstNoOp(name=f"I-wsplit-{ctr}", ins=[], outs=[])
                        ctr += 1
                        nop.engine = ins.engine
                        nop.sync_info = mybir.SyncInfo(on_wait=[w], on_update=[])
                        nc.register_instruction(nop)
                        rewritten.append(nop)
                    ins.sync_info = mybir.SyncInfo(
                        on_wait=[waits[-1]], on_update=list(si.on_update)
                    )
                    changed = True
                rewritten.append(ins)
            if changed:
                blk.instructions = rewritten
    return nc


F32 = mybir.dt.float32
BF16 = mybir.dt.bfloat16
BF16_NP = ml_dtypes.bfloat16
ADD = mybir.AluOpType.add
MULT = mybir.AluOpType.mult
EXP = mybir.ActivationFunctionType.Exp

P = 128
D_MODEL = 1024
N_HEADS = 16
HEAD_DIM = 64
SCALE = HEAD_DIM**-0.5

NL = 256  # local projection dims (4 heads x 64)
HL = 4  # local heads
QBS = 512  # q block size for attention
LAG = 3  # steps PV trails the scores/exp pipeline


def build_bass(S: int) -> bass.Bass:
    """One SPMD program; every core runs it on its own shard."""
    D = D_MODEL
    DC = D // P  # 8
    SC = S // P  # 16
    KC = S // P  # 16
    QB = S // QBS  # 4

    nc = bass.Bass()
    # all inputs host-prearranged to partition-major SBUF layouts so every
    # DMA is 128 descriptors of >=2KB contiguous per partition.
    xt = nc.declare_dram_parameter("xt", [P, QB, DC, 512], BF16, isOutput=False)
    wq = nc.declare_dram_parameter("wq", [P, DC, NL], BF16, isOutput=False)
    wk = nc.declare_dram_parameter("wk", [P, DC, NL], BF16, isOutput=False)
    wv = nc.declare_dram_parameter("wv", [P, DC, NL], BF16, isOutput=False)
    bq = nc.declare_dram_parameter("bq", [NL], F32, isOutput=False)
    bk = nc.declare_dram_parameter("bk", [NL], F32, isOutput=False)
    bv = nc.declare_dram_parameter("bv", [NL], F32, isOutput=False)
    wo = nc.declare_dram_parameter("wo", [P, 2, D], BF16, isOutput=False)
    y = nc.declare_dram_parameter("y", [S, D], BF16, isOutput=True)

    with TileContext(nc) as tc:
        with (
            tc.tile_pool(name="persist", bufs=1) as pp,
            tc.tile_pool(name="small", bufs=3) as small,
            tc.tile_pool(name="psum", bufs=1, space="PSUM") as psp,
        ):
            # ---- constants / ACT table warm-up ----
            ones = pp.tile([P, HEAD_DIM], BF16, name="ones")
            nc.vector.memset(ones, 1.0)
            warm_in = pp.tile([P, 1], F32, name="warm_in")
            nc.vector.memset(warm_in, 0.0)
            warm_out = pp.tile([P, 1], F32, name="warm_out")
            nc.scalar.activation(warm_out, warm_in, EXP)
            # HAM warm-up: dummy matmuls during the input-DMA wait so the
            # first real projection pieces run at the full 2.4GHz clock.
            junk = pp.tile([P, 512], BF16, name="junk")
            nc.vector.memset(junk, 0.0)
            ps_w = psp.tile([P, 512], F32, tag="gen", bufs=2, name="ps_warm")
            for i in range(24):
                nc.tensor.matmul(
                    ps_w[0:HEAD_DIM], lhsT=ones, rhs=junk,
                    start=(i == 0), stop=(i == 23),
                )

            # ---- persistent activations ----
            xT = pp.tile([P, QB, DC, 512], BF16, name="xT")  # s-block major
            QT = pp.tile([P, 2, S], BF16, name="QT")
            KT = pp.tile([P, 2, S], BF16, name="KT")
            V = pp.tile([P, SC, HL, HEAD_DIM], BF16, name="V")
            outT = pp.tile([P, 2, S], BF16, name="outT")
            expT = pp.tile([P, KC, 2, QBS], BF16, name="expT")

            wq_sb = pp.tile([P, DC, NL], BF16, name="wq_sb")
            wk_sb = pp.tile([P, DC, NL], BF16, name="wk_sb")
            wv_sb = pp.tile([P, DC, NL], BF16, name="wv_sb")
            wo_sb = pp.tile([P, 2, D], BF16, name="wo_sb")
            bq_sb = pp.tile([P, 2], F32, name="bq_sb")
            bk_sb = pp.tile([P, 2], F32, name="bk_sb")
            bv_bc = pp.tile([P, NL], F32, name="bv_bc")

            # x s-block 0 split across BOTH rings so the first projection
            # pieces unblock earliest; sync ring (fast) carries the rest of x.
            nc.sync.dma_start(xT[:, 0, 0:2], xt[:, 0, 0:2])
            nc.sync.dma_start(wk_sb, wk[:])
            nc.sync.dma_start(wq_sb, wq[:])
            nc.sync.dma_start(bq_sb, bq[:].rearrange("(o p) -> p o", p=P))
            nc.sync.dma_start(bk_sb, bk[:].rearrange("(o p) -> p o", p=P))
            nc.sync.dma_start(bv_bc, bv[:].unsqueeze(0).to_broadcast((P, NL)))
            nc.sync.dma_start(xT[:, 1], xt[:, 1])
            nc.sync.dma_start(xT[:, 3], xt[:, 3])
            nc.scalar.dma_start(xT[:, 0, 2:4], xt[:, 0, 2:4])
            nc.scalar.dma_start(xT[:, 0, 4:8], xt[:, 0, 4:8])
            nc.scalar.dma_start(wv_sb, wv[:])
            nc.scalar.dma_start(xT[:, 2], xt[:, 2])
            nc.scalar.dma_start(wo_sb, wo[:])

            # ---- projection / out-proj pieces (PE filler units) ----
            def proj_part(w_sb, b_sb, dest, nsub, sb, state, dcs):
                if "ps" not in state:
                    state["ps"] = psp.tile([P, 512], F32, tag="gen", bufs=2, name="ps_p")
                ps = state["ps"]
                for dc in dcs:
                    nc.tensor.matmul(
                        ps,
                        lhsT=w_sb[:, dc, nsub * P : (nsub + 1) * P],
                        rhs=xT[:, sb, dc, :],
                        start=(dc == 0),
                        stop=(dc == DC - 1),
                    )
                if dcs[-1] == DC - 1:
                    nc.vector.tensor_scalar(
                        dest[:, nsub, sb * 512 : (sb + 1) * 512],
                        ps,
                        b_sb[:, nsub : nsub + 1],
                        None,
                        ADD,
                    )

            def proj_piece(w_sb, b_sb, dest, nsub, sb):
                proj_part(w_sb, b_sb, dest, nsub, sb, {}, list(range(DC)))

            def v_piece(sc):
                ps = psp.tile([P, 512], F32, tag="gen", bufs=2, name="ps_v")
                psv = ps[:, :NL]
                for dc in range(DC):
                    nc.tensor.matmul(
                        psv,
                        lhsT=xT[:, sc // 4, dc, (sc % 4) * P : (sc % 4 + 1) * P],
                        rhs=wv_sb[:, dc, :],
                        start=(dc == 0),
                        stop=(dc == DC - 1),
                    )
                nc.vector.tensor_tensor(
                    V[:, sc],
                    psv.rearrange("p (h d) -> p h d", h=HL),
                    bv_bc.rearrange("p (h d) -> p h d", h=HL),
                    ADD,
                )

            def y_piece(qc, mb, tail=False):
                ps = psp.tile([P, 512], F32, tag="gen", bufs=2, name="ps_y")
                for nch in range(2):
                    nc.tensor.matmul(
                        ps,
                        lhsT=outT[:, nch, qc * P : (qc + 1) * P],
                        rhs=wo_sb[:, nch, mb * 512 : (mb + 1) * 512],
                        start=(nch == 0),
                        stop=(nch == 1),
                    )
                yt = small.tile([P, 512], BF16, tag="yt")
                # in the drain the ACT engine is free: splitting the casts
                # across DVE + ACT halves the serial cast chain there.
                if tail and mb == 1:
                    nc.scalar.copy(yt, ps)
                else:
                    nc.vector.tensor_copy(yt, ps)
                nc.sync.dma_start(
                    y[qc * P : (qc + 1) * P, mb * 512 : (mb + 1) * 512], yt
                )

            # ---- weave schedule: step index -> filler closures ----
            fill: dict[int, list] = {}

            def put(s, fn):
                fill.setdefault(s, []).append(fn)

            def put_proj(s, w_sb, b_sb, dest, nsub, sb):
                # split into two 4-matmul halves on adjacent steps so a weave
                # never delays the next score pair beyond the ACT lookahead
                state: dict = {}
                put(s, lambda: proj_part(w_sb, b_sb, dest, nsub, sb, state, [0, 1, 2, 3]))
                put(s + 1, lambda: proj_part(w_sb, b_sb, dest, nsub, sb, state, [4, 5, 6, 7]))

            for kc in range(16):  # V just-in-time for block 0's PV (lag 3)
                put(kc + 1, lambda sc=kc: v_piece(sc))
            put_proj(2, wk_sb, bk_sb, KT, 0, 1)
            put_proj(6, wk_sb, bk_sb, KT, 0, 2)
            put_proj(10, wk_sb, bk_sb, KT, 0, 3)
            put_proj(12, wq_sb, bq_sb, QT, 0, 1)  # needed by t=1 (qb1, hp0)
            # block 1 carries the ns1 projections (needed from t=2 on)
            put_proj(17, wk_sb, bk_sb, KT, 1, 0)
            put_proj(19, wk_sb, bk_sb, KT, 1, 1)
            put_proj(22, wk_sb, bk_sb, KT, 1, 2)
            put_proj(25, wk_sb, bk_sb, KT, 1, 3)
            put_proj(28, wq_sb, bq_sb, QT, 1, 0)  # deadline t=2 (step 32)
            put_proj(33, wq_sb, bq_sb, QT, 1, 1)  # deadline t=3 (step 48)
            put_proj(49, wq_sb, bq_sb, QT, 0, 2)  # deadline t=4 (step 64)
            put_proj(76, wq_sb, bq_sb, QT, 1, 2)  # deadline t=5 (step 80)
            put_proj(90, wq_sb, bq_sb, QT, 0, 3)  # deadline t=6 (step 96)
            put_proj(97, wq_sb, bq_sb, QT, 1, 3)  # deadline t=7 (step 112)
            # y(qb) needs both (qb, hp0) and (qb, hp1) normalized:
            # q0 after t=2 (~step 53), q1 after t=3 (~step 69), q2 after t=5
            for j, base in ((0, 56), (1, 72), (2, 104)):
                for k in range(8):
                    put(base + 2 * k, lambda qc=4 * j + k // 2, mb=k % 2: y_piece(qc, mb))

            # ---- flat 128-step stream ----
            # paired-qb order: block 0 only needs the ns0 projections, and
            # each qb's two hp blocks finish within 3 blocks of each other.
            blocks = [(0, 0), (1, 0), (0, 1), (1, 1), (2, 0), (2, 1), (3, 0), (3, 1)]
            steps = [(t, kc) for t in range(len(blocks)) for kc in range(KC)]
            pv_tiles: dict = {}

            def emit_pv(ls):
                lt, lkc = steps[ls]
                lqb, lhp = blocks[lt]
                if lt not in pv_tiles:
                    pv_tiles[lt] = (
                        psp.tile([P, QBS], F32, tag="pv", bufs=1, name="pv"),
                        psp.tile([P, QBS], F32, tag="sum", bufs=1, name="sm"),
                    )
                pv, sm = pv_tiles[lt]
                st, sp = (lkc == 0), (lkc == KC - 1)
                nc.tensor.matmul(
                    pv[0:HEAD_DIM],
                    lhsT=V[:, lkc, 2 * lhp, :],
                    rhs=expT[:, lkc, 0, :],
                    start=st,
                    stop=sp,
                    skip_group_check=True,
                    tile_position=(0, 0),
                )
                nc.tensor.matmul(
                    pv[HEAD_DIM:P],
                    lhsT=V[:, lkc, 2 * lhp + 1, :],
                    rhs=expT[:, lkc, 1, :],
                    start=st,
                    stop=sp,
                    skip_group_check=True,
                    tile_position=(0, 64),
                )
                nc.tensor.matmul(
                    sm[0:HEAD_DIM],
                    lhsT=ones,
                    rhs=expT[:, lkc, 0, :],
                    start=st,
                    stop=sp,
                    skip_group_check=True,
                    tile_position=(0, 0),
                )
                nc.tensor.matmul(
                    sm[HEAD_DIM:P],
                    lhsT=ones,
                    rhs=expT[:, lkc, 1, :],
                    start=st,
                    stop=sp,
                    skip_group_check=True,
                    tile_position=(0, 64),
                )
                if lkc == KC - 1:
                    finish_block(lt)

            def finish_block(t):
                qb, hp = blocks[t]
                pv, sm = pv_tiles.pop(t)
                pvs = small.tile([P, QBS], F32, tag="pvs")
                nc.vector.tensor_copy(pvs, pv)
                smsb = small.tile([P, QBS], F32, tag="smsb")
                nc.vector.tensor_copy(smsb, sm)
                rbc = small.tile([P, QBS], F32, tag="rbc")
                if t < len(blocks) - 1:
                    nc.vector.reciprocal(rbc, smsb)
                    nc.vector.tensor_tensor(
                        outT[:, hp, qb * QBS : (qb + 1) * QBS], pvs, rbc, MULT
                    )
                else:
                    # last block: normalize in quarters, interleaving the
                    # final out-proj pieces, so the tail drain stays short
                    # and the PE never idles into a HAM re-throttle.
                    # warm-keeper dummies go in the score-psum ring (now
                    # idle); the gen ring must stay free for the y pieces.
                    warm_s = psp.tile([P, 2, QBS], F32, tag="s", bufs=2, name="ps_tailwarm")
                    for h in range(4):
                        sl = slice(h * 128, (h + 1) * 128)
                        nc.vector.reciprocal(rbc[:, sl], smsb[:, sl])
                        nc.vector.tensor_tensor(
                            outT[:, hp, qb * QBS + h * 128 : qb * QBS + (h + 1) * 128],
                            pvs[:, sl],
                            rbc[:, sl],
                            MULT,
                        )
                        nc.tensor.matmul(
                            warm_s[0:HEAD_DIM, 0], lhsT=ones, rhs=junk,
                            start=(h == 0), stop=False,
                        )
                        for mb in range(2):
                            y_piece(4 * qb + h, mb, tail=True)
                        nc.tensor.matmul(
                            warm_s[0:HEAD_DIM, 0], lhsT=ones, rhs=junk,
                            start=False, stop=(h == 3),
                        )

            # phase A: first KT piece + first QT piece gate the stream
            proj_piece(wk_sb, bk_sb, KT, 0, 0)
            proj_piece(wq_sb, bq_sb, QT, 0, 0)

            for s, (t, kc) in enumerate(steps):
                qb, hp = blocks[t]
                for fn in fill.get(s, ()):
                    fn()
                ps = psp.tile([P, 2, QBS], F32, tag="s", bufs=2, name="ps_s")
                nc.tensor.matmul(
                    ps[:, 0],
                    lhsT=KT[0:HEAD_DIM, hp, kc * P : (kc + 1) * P],
                    rhs=QT[0:HEAD_DIM, hp, qb * QBS : (qb + 1) * QBS],
                    start=True,
                    stop=True,
                )
                nc.tensor.matmul(
                    ps[:, 1],
                    lhsT=KT[HEAD_DIM:P, hp, kc * P : (kc + 1) * P],
                    rhs=QT[HEAD_DIM:P, hp, qb * QBS : (qb + 1) * QBS],
                    start=True,
                    stop=True,
                )
                nc.scalar.activation(expT[:, kc], ps, EXP, scale=SCALE)
                if s >= LAG:
                    emit_pv(s - LAG)
            for ls in range(len(steps) - LAG, len(steps)):
                emit_pv(ls)

    _split_excess_waits(nc)
    return nc


def _w_pmajor(W):
    """[D, NL] -> [128, DC, NL] partition-major (p = d % 128, dc = d // 128)."""
    D, n = W.shape
    return np.ascontiguousarray(
        W.reshape(D // 128, 128, n).transpose(1, 0, 2)
    ).astype(BF16_NP)


def shard_inputs(x, Wq, bq, Wk, bk, Wv, bv, Wo, bo):
    """Split full inputs into 8 per-core maps: core c -> (batch c//4, head
    group c%4).  x is transposed, bf16-converted, AND prearranged to the
    SBUF layout [p, s_block, dc, s'] on the host so device DMAs are 128
    descriptors of 8KB contiguous."""
    B, S, D = x.shape
    xts = []
    for b in range(B):
        # x[b] [S, D] -> xT [D, S] -> [dc, p, sb, s'] -> [p, sb, dc, s']
        xt = x[b].T.reshape(D // 128, 128, S // 512, 512).transpose(1, 2, 0, 3)
        xts.append(np.ascontiguousarray(xt).astype(BF16_NP))
    in_maps = []
    for c in range(8):
        b, g = c // 4, c % 4
        n0 = g * NL
        in_maps.append(
            {
                "xt": xts[b],
                "wq": _w_pmajor(Wq[:, n0 : n0 + NL]),
                "wk": _w_pmajor(Wk[:, n0 : n0 + NL]),
                "wv": _w_pmajor(Wv[:, n0 : n0 + NL]),
                "bq": np.ascontiguousarray(bq[n0 : n0 + NL]),
                "bk": np.ascontiguousarray(bk[n0 : n0 + NL]),
                "bv": np.ascontiguousarray(bv[n0 : n0 + NL]),
                "wo": _w_pmajor(Wo[n0 : n0 + NL, :]),
            }
        )
    return in_maps


_NC_CACHE = {}


def kernel(x, Wq, bq, Wk, bk, Wv, bv, Wo, bo, trace=False, tmpdir=None):
    from concourse.bass_utils import run_bass_kernel_spmd

    x = np.asarray(x, dtype=np.float32)
    args = [np.asarray(a, dtype=np.float32) for a in (Wq, bq, Wk, bk, Wv, bv, Wo, bo)]
    B, S, D = x.shape

    if S not in _NC_CACHE:
        _NC_CACHE[S] = build_bass(S)
    nc = _NC_CACHE[S]

    in_maps = shard_inputs(x, *args)
    res = run_bass_kernel_spmd(
        nc, in_maps, core_ids=list(range(8)), trace=trace, tmpdir=tmpdir
    )
    if trace:
        kernel.last_result = res
    bo_f = args[7]
    parts = [np.asarray(res.results[c]["y"]).astype(np.float32) for c in range(8)]
    out = np.empty((B, S, D), dtype=np.float32)
    for b in range(B):
        out[b] = parts[4 * b] + parts[4 * b + 1] + parts[4 * b + 2] + parts[4 * b + 3]
        out[b] += bo_f
    return out
